# revision 8
# baseline (speedup 1.0000x reference)
"""Trainium2 Bass kernel for nn_Mesh_Renderer: silhouette via scanline intervals.

Data-parallel over batch (core b renders view b). Host work is layout only
(gather vertices[faces], constant tables, transpose the returned image). All
input-dependent math on device.

Device algorithm (per core):
  1. Camera basis computed with one axis per partition ([3, *] tiles) so the
     projection matrix rt4 = [R^T; -R@eye] [4, 3] is produced by a single PE
     transpose (no SBUF reshape DMA). Projection: 40 K=4 f32 matmuls
     vca[f, (ft, k, d)], perspective divide -> per-corner (xn, yn).
  2. Edge coefficients -> per-edge t-planes t = u*y + v: (u, v) for BOTH the
     lower and negated-upper side packed in one [128, 120] tile, one PE
     transpose -> bf16 lhsT, evaluated against the block-diagonal tbasis by
     PE; drains via ACT (lo) and Pool (nh). Row interval [lo(i), hi(i)] per
     face via DVE max chains; invisible/degenerate faces forced to far-away
     point intervals with +-BIG offsets (sign-sum contributes 0).
  3. Raster, count formulation: cnt(px) = sum_f w_f([x >= lo] + [-x >= -hi]).
     Face tiles split across three pipelines to balance engines:
       L: one DVE is_ge [128, 8192] bf16 -> 16 PE accum matmuls (weight 2).
       S: PE diff-plane matmuls (x - lo, hi - x vs constant xb65 basis) ->
          ACT Sign -> fp8 masks (sign-sum in {0,1,2}, weight 1).
       CV: DVE is_ge -> Pool converts the bf16 masks to fp8 (weight 2).
     fp8 tiles are paired and accumulated with fp8 DoubleRow matmuls
     (2 face tiles per pass, 0.5 cyc/row) into the same PSUM cnt [8, 512].
  4. silhouette = cnt > 2*128*n_step_tiles + 0.5; two-half DMA out; host
     transposes (j, i) -> (i, j).
"""

import sys

if "/opt/trn_rl_repo" not in sys.path:
    sys.path.insert(0, "/opt/trn_rl_repo")

import ml_dtypes
import numpy as np

import concourse.bacc as bacc
import concourse.tile as tile
from concourse import mybir
from concourse.bass_utils import run_bass_kernel_spmd

F32 = mybir.dt.float32
BF16 = mybir.dt.bfloat16
FP8 = mybir.dt.float8e4
I32 = mybir.dt.int32
OP = mybir.AluOpType
AF = mybir.ActivationFunctionType

B, V, NF, IMG = 8, 642, 1280, 64
NPIX = IMG * IMG          # 4096
NTILE = NF // 128         # 10 face tiles
EPS = 1e-8
BIG = 1.0e30
TAN_T = float(np.tan(np.deg2rad(np.float32(15.0)).astype(np.float32)))

# raster pipeline assignment per face tile
S_TILES = (3, 6, 9)          # PE diff + ACT sign -> fp8
CV_TILES = (0,)              # DVE compare + Pool bf16->fp8 convert
PAIRS = ((3, 6), (9, 0))     # DoubleRow pairs (halves in emission order)
L_TILES = tuple(t for t in range(NTILE)
                if t not in S_TILES and t not in CV_TILES)
N_STEP_FACES = 128 * (len(L_TILES) + len(CV_TILES))
THR = 2.0 * N_STEP_FACES + 0.5


def build_kernel(ctx, tc):
    nc = tc.nc
    eye_d = nc.dram_tensor("eye3", [3, 3], F32, kind="ExternalInput")
    vgt_d = nc.dram_tensor("vgt4", [4, 4 * NF], F32, kind="ExternalInput")
    cb_d = nc.dram_tensor("cblob", [128, 132], F32, kind="ExternalInput")
    ohb_d = nc.dram_tensor("ohb", [128, 16], BF16, kind="ExternalInput")
    oh8_d = nc.dram_tensor("oh8", [128, 64], FP8, kind="ExternalInput")
    xb_d = nc.dram_tensor("xb65", [65, NPIX], BF16, kind="ExternalInput")
    tb_d = nc.dram_tensor("tbasis", [60, 1920], BF16, kind="ExternalInput")
    xg_d = nc.dram_tensor("xgrid", [128, 2 * NPIX], BF16, kind="ExternalInput")
    sil_d = nc.dram_tensor("sil", [NPIX], F32, kind="ExternalOutput")

    cpool = ctx.enter_context(tc.tile_pool(name="cam", bufs=1))
    ppool = ctx.enter_context(tc.tile_pool(name="proj", bufs=1))
    gpool = ctx.enter_context(tc.tile_pool(name="grid", bufs=1))

    # ---- input DMAs (emission order = DMA_ENGINES order; big xgrid early
    # but behind everything the compute spine needs) ----
    eye3 = cpool.tile([3, 3], F32)
    nc.sync.dma_start(eye3[:], eye_d.ap())
    vgt = gpool.tile([4, 4 * NF], F32)
    nc.sync.dma_start(vgt[:], vgt_d.ap())
    cb = gpool.tile([128, 132], F32)
    nc.sync.dma_start(cb[:], cb_d.ap())
    ohb = gpool.tile([128, 16], BF16)
    nc.sync.dma_start(ohb[:], ohb_d.ap())
    oh8 = gpool.tile([128, 64], FP8)
    nc.sync.dma_start(oh8[:], oh8_d.ap())
    xb65 = gpool.tile([65, NPIX], BF16)
    nc.sync.dma_start(xb65[:], xb_d.ap())
    tb = gpool.tile([60, 1920], BF16)
    nc.sync.dma_start(tb[:], tb_d.ap())
    xx = gpool.tile([128, 2 * NPIX], BF16)
    nc.sync.dma_start(xx[:], xg_d.ap())
    xxv = xx[:].rearrange("p (s j i) -> p s j i", s=2, j=IMG)

    idm = cb[:, 0:128]                    # f32 identity
    idmb = gpool.tile([128, 128], BF16)
    nc.vector.tensor_copy(idmb[:], idm)

    # ---- camera basis, one axis per partition: p0=x, p1=y, p2=z ----
    nege = cpool.tile([3, 3], F32)
    nc.vector.tensor_scalar(nege[:], eye3[:], -1.0, None, OP.mult)
    # xr = cross(up, -eye) = (nege_z, 0, -nege_x)
    xr = cpool.tile([3, 3], F32)
    nc.vector.memset(xr[:], 0.0)
    nc.vector.tensor_copy(xr[:, 0:1], nege[:, 2:3])
    nc.vector.tensor_scalar(xr[:, 2:3], nege[:, 0:1], -1.0, None, OP.mult)
    # normalize x (DVE) and z (Pool/ACT) in parallel; duplicated [3, 6]
    sqx = cpool.tile([3, 3], F32)
    nc.vector.tensor_tensor(sqx[:], xr[:], xr[:], OP.mult)
    ssx = cpool.tile([3, 1], F32)
    nc.vector.tensor_reduce(ssx[:], sqx[:], mybir.AxisListType.X, OP.add)
    nx_ = cpool.tile([3, 1], F32)
    nc.scalar.activation(nx_[:], ssx[:], AF.Sqrt)
    rx_ = cpool.tile([3, 1], F32)
    nc.vector.reciprocal(rx_[:], nx_[:])
    xn = cpool.tile([3, 6], F32)
    nc.vector.tensor_scalar(
        xn[:].rearrange("p (two d) -> p two d", d=3),
        xr[:].unsqueeze(1).broadcast_to([3, 2, 3]), rx_[:], None, OP.mult)
    sqz = cpool.tile([3, 3], F32)
    nc.gpsimd.tensor_tensor(sqz[:], nege[:], nege[:], OP.mult)
    ssz1 = cpool.tile([3, 1], F32)
    nc.gpsimd.tensor_tensor(ssz1[:], sqz[:, 0:1], sqz[:, 1:2], OP.add)
    ssz = cpool.tile([3, 1], F32)
    nc.gpsimd.tensor_tensor(ssz[:], ssz1[:], sqz[:, 2:3], OP.add)
    nz_ = cpool.tile([3, 1], F32)
    nc.scalar.activation(nz_[:], ssz[:], AF.Sqrt)
    rz_ = cpool.tile([3, 1], F32)
    nc.vector.reciprocal(rz_[:], nz_[:])
    zn = cpool.tile([3, 6], F32)
    nc.vector.tensor_scalar(
        zn[:].rearrange("p (two d) -> p two d", d=3),
        nege[:].unsqueeze(1).broadcast_to([3, 2, 3]), rz_[:], None, OP.mult)
    # y = cross(z, x)
    m1 = cpool.tile([3, 3], F32)
    nc.vector.tensor_tensor(m1[:], zn[:, 1:4], xn[:, 2:5], OP.mult)
    m2 = cpool.tile([3, 3], F32)
    nc.vector.tensor_tensor(m2[:], zn[:, 2:5], xn[:, 1:4], OP.mult)
    y0 = cpool.tile([3, 3], F32)
    nc.vector.tensor_tensor(y0[:], m1[:], m2[:], OP.subtract)
    # per-partition axis select: axes[p] = x*[p==0] + y*[p==1] + z*[p==2]
    t1 = cpool.tile([3, 3], F32)
    nc.vector.tensor_scalar(t1[:], xn[:, 0:3], cb[0:3, 128:129], None, OP.mult)
    t2 = cpool.tile([3, 3], F32)
    nc.vector.tensor_scalar(t2[:], y0[:], cb[0:3, 129:130], None, OP.mult)
    t3 = cpool.tile([3, 3], F32)
    nc.vector.tensor_scalar(t3[:], zn[:, 0:3], cb[0:3, 130:131], None, OP.mult)
    ax12 = cpool.tile([3, 3], F32)
    nc.vector.tensor_tensor(ax12[:], t1[:], t2[:], OP.add)
    axes = cpool.tile([3, 3], F32)
    nc.vector.tensor_tensor(axes[:], ax12[:], t3[:], OP.add)
    # nreye[p] = -dot(eye, axis_p)
    el = cpool.tile([3, 3], F32)
    nc.vector.tensor_tensor(el[:], eye3[:], axes[:], OP.mult)
    dt_ = cpool.tile([3, 1], F32)
    nc.vector.tensor_reduce(dt_[:], el[:], mybir.AxisListType.X, OP.add)
    # rt4T [3, 4]: rows are [axis | -dot]
    rt4T = cpool.tile([3, 4], F32)
    nc.vector.tensor_copy(rt4T[:, 0:3], axes[:])
    nc.vector.tensor_scalar(rt4T[:, 3:4], dt_[:], -1.0, None, OP.mult)
    rt4 = cpool.tile([4, 3], F32)
    with tc.tile_pool(name="ptc", bufs=1, space="PSUM") as ptc:
        rt4p = ptc.tile([4, 3], F32, tag="rt4p")
        nc.tensor.transpose(rt4p[:], rt4T[:], idm[0:3, 0:3])
        nc.vector.tensor_copy(rt4[:], rt4p[:])

    # ---- projection: vca[p, (ft, k, d)] = [w;1]^T @ rt4 per corner ----
    vca = ppool.tile([128, 120], F32)
    with tc.tile_pool(name="pvc", bufs=1, space="PSUM") as psvc:
        vcp = psvc.tile([128, 120], F32)
        for ft in range(NTILE):
            for k in range(4):
                nc.tensor.matmul(
                    vcp[:, 12 * ft + 3 * k : 12 * ft + 3 * (k + 1)],
                    vgt[:, NF * k + 128 * ft : NF * k + 128 * (ft + 1)],
                    rt4[:],
                    start=True,
                    stop=True,
                )
        nc.vector.tensor_copy(vca[:], vcp[:])

    vcav = vca[:].rearrange("p (c d) -> p c d", d=3)
    vx, vy, vz = vcav[:, :, 0], vcav[:, :, 1], vcav[:, :, 2]

    # perspective divide (raw reciprocal; interval margins tolerate ~3e-3)
    dn = ppool.tile([128, 40], F32)
    nc.vector.tensor_scalar(dn[:], vz, TAN_T, EPS, OP.mult, OP.add)
    rc = ppool.tile([128, 40], F32)
    nc.vector.reciprocal(rc[:], dn[:])
    xn_ = ppool.tile([128, 40], F32)
    nc.vector.tensor_tensor(xn_[:], vx, rc[:], OP.mult)
    yn_ = ppool.tile([128, 40], F32)
    nc.vector.tensor_tensor(yn_[:], vy, rc[:], OP.mult)

    # visibility: all corner z > 0 (on Pool)
    vz4 = vca[:].rearrange("p (ft k d) -> p ft k d", k=4, d=3)
    mz1 = ppool.tile([128, 10], F32)
    nc.vector.tensor_tensor(mz1[:], vz4[:, :, 0, 2], vz4[:, :, 1, 2], OP.min)
    mz = ppool.tile([128, 10], F32)
    nc.vector.tensor_tensor(mz[:], mz1[:], vz4[:, :, 2, 2], OP.min)
    vg = ppool.tile([128, 10], F32)
    nc.vector.tensor_scalar(vg[:], mz[:], 0.0, None, OP.is_gt)

    # ---- edge coefficients [128, 30] in (ft, k) layout ----
    xn4 = xn_[:].rearrange("p (ft k) -> p ft k", k=4)
    yn4 = yn_[:].rearrange("p (ft k) -> p ft k", k=4)
    xk, xk1 = xn4[:, :, 0:3], xn4[:, :, 1:4]
    yk, yk1 = yn4[:, :, 0:3], yn4[:, :, 1:4]

    def t30(name):
        return ppool.tile([128, 30], F32, name=name, tag=name)

    A = t30("A")
    Av = A[:].rearrange("p (ft k) -> p ft k", k=3)
    nc.vector.tensor_tensor(Av, yk, yk1, OP.subtract)
    Bc = t30("Bc")
    Bv = Bc[:].rearrange("p (ft k) -> p ft k", k=3)
    nc.vector.tensor_tensor(Bv, xk1, xk, OP.subtract)
    p1 = t30("p1")
    nc.gpsimd.tensor_tensor(p1[:].rearrange("p (ft k) -> p ft k", k=3), xk,
                            yk1, OP.mult)
    p2 = t30("p2")
    nc.gpsimd.tensor_tensor(p2[:].rearrange("p (ft k) -> p ft k", k=3), yk,
                            xk1, OP.mult)
    C = t30("C")
    nc.gpsimd.tensor_tensor(C[:], p1[:], p2[:], OP.subtract)

    Cv = C[:].rearrange("p (ft k) -> p ft k", k=3)
    S1 = ppool.tile([128, 10], F32, name="S1")
    nc.gpsimd.tensor_tensor(S1[:], Cv[:, :, 0], Cv[:, :, 1], OP.add)
    S = ppool.tile([128, 10], F32, name="S")
    nc.gpsimd.tensor_tensor(S[:], S1[:], Cv[:, :, 2], OP.add)

    # masks (Pool side-chain)
    w = t30("w")
    nc.gpsimd.tensor_tensor(w[:].rearrange("p (ft k) -> p ft k", k=3), Av,
                            S[:].unsqueeze(2).broadcast_to([128, 10, 3]),
                            OP.mult)
    # reciprocal side (DVE)
    iseq = t30("iseq")
    nc.vector.tensor_scalar(iseq[:], A[:], 0.0, None, OP.is_equal)
    Asafe = t30("Asafe")
    nc.vector.tensor_tensor(Asafe[:], A[:], iseq[:], OP.add)
    r0 = t30("r0")
    nc.vector.reciprocal(r0[:], Asafe[:])
    nr = t30("nr")
    nc.vector.tensor_scalar(nr[:], r0[:], -1.0, None, OP.mult)
    u = t30("u")
    nc.vector.tensor_tensor(u[:], Bc[:], nr[:], OP.mult)
    v = t30("v")
    nc.vector.tensor_tensor(v[:], C[:], nr[:], OP.mult)
    mpos = t30("mpos")
    nc.vector.tensor_scalar(mpos[:], w[:], 0.0, None, OP.is_gt)
    mneg = t30("mneg")
    nc.vector.tensor_scalar(mneg[:], w[:], 0.0, None, OP.is_lt)
    offlo = t30("offlo")
    nc.vector.tensor_scalar(offlo[:], mpos[:], BIG, -BIG, OP.mult, OP.add)
    offnh = t30("offnh")
    nc.vector.tensor_scalar(offnh[:], mneg[:], BIG, -BIG, OP.mult, OP.add)
    mnegN = t30("mnegN")
    nc.vector.tensor_scalar(mnegN[:], mneg[:], -1.0, None, OP.mult)

    sne = ppool.tile([128, 10], F32, name="sne")
    nc.vector.tensor_scalar(sne[:], S[:], 0.0, None, OP.not_equal)
    visq = ppool.tile([128, 10], F32, name="visq")
    nc.gpsimd.tensor_tensor(visq[:], vg[:], sne[:], OP.mult)
    ivq = ppool.tile([128, 10], F32, name="ivq")
    nc.vector.tensor_scalar(ivq[:], visq[:], -2.0 * BIG, 2.0 * BIG, OP.mult,
                            OP.add)
    ivqN = ppool.tile([128, 10], F32, name="ivqN")
    nc.vector.tensor_scalar(ivqN[:], visq[:], 2.0 * BIG, -2.0 * BIG, OP.mult,
                            OP.add)

    # (u, v) staging for BOTH sides in one [128, 120] tile: cols 0..60 the
    # lower side, 60..120 the negated-upper side, each (m, 2) = (u, v)
    uv12 = ppool.tile([128, 120], F32, name="uv12")
    uvlov = uv12[:, 0:60].rearrange("p (m two) -> p m two", two=2)
    uvnhv = uv12[:, 60:120].rearrange("p (m two) -> p m two", two=2)

    # lower side: ulo = u*mpos ; vlo = v*mpos + offlo + ivq
    nc.vector.tensor_tensor(uvlov[:, :, 0], u[:], mpos[:], OP.mult)
    vlo1 = t30("vlo1")
    nc.vector.tensor_tensor(vlo1[:], v[:], mpos[:], OP.mult)
    vlo2 = t30("vlo2")
    nc.vector.tensor_tensor(vlo2[:], vlo1[:], offlo[:], OP.add)
    nc.vector.tensor_tensor(
        uvlov[:, :, 1].rearrange("p (ft k) -> p ft k", k=3),
        vlo2[:].rearrange("p (ft k) -> p ft k", k=3),
        ivq[:].unsqueeze(2).broadcast_to([128, 10, 3]), OP.add)

    # negated upper side: unh = -u*mneg ; vnh = -v*mneg + offnh - ivq
    nc.vector.tensor_tensor(uvnhv[:, :, 0], u[:], mnegN[:], OP.mult)
    vnh1 = t30("vnh1")
    nc.vector.tensor_tensor(vnh1[:], v[:], mnegN[:], OP.mult)
    vnh2 = t30("vnh2")
    nc.vector.tensor_tensor(vnh2[:], vnh1[:], offnh[:], OP.add)
    nc.vector.tensor_tensor(
        uvnhv[:, :, 1].rearrange("p (ft k) -> p ft k", k=3),
        vnh2[:].rearrange("p (ft k) -> p ft k", k=3),
        ivqN[:].unsqueeze(2).broadcast_to([128, 10, 3]), OP.add)

    # ---- T planes via PE for both sides; drains: lo on ACT, nh on Pool ----
    uvloB = gpool.tile([60, 128], BF16)
    uvnhB = gpool.tile([60, 128], BF16)
    with tc.tile_pool(name="ptr", bufs=2, space="PSUM") as ptr:
        uvloT = ptr.tile([60, 128], F32, tag="uvT")
        nc.tensor.transpose(uvloT[:], uv12[:, 0:60], idm)
        nc.scalar.activation(uvloB[:], uvloT[:], AF.Copy)
        uvnhT = ptr.tile([60, 128], F32, tag="uvT")
        nc.tensor.transpose(uvnhT[:], uv12[:, 60:120], idm)
        nc.scalar.activation(uvnhB[:], uvnhT[:], AF.Copy)
    TLOs = gpool.tile([128, 1920], BF16)
    TNHs = gpool.tile([128, 1920], BF16)
    with tc.tile_pool(name="ptp", bufs=2, space="PSUM") as ptp:
        for h in range(2):
            TLOp = ptp.tile([128, 960], F32, tag="tp")
            for q in range(2):
                nc.tensor.matmul(
                    TLOp[:, 480 * q : 480 * (q + 1)], uvloB[:],
                    tb[:, 960 * h + 480 * q : 960 * h + 480 * (q + 1)],
                    start=True, stop=True)
            nc.scalar.activation(TLOs[:, 960 * h : 960 * (h + 1)], TLOp[:],
                                 AF.Copy)
        for h in range(2):
            TNHp = ptp.tile([128, 960], F32, tag="tp")
            for q in range(2):
                nc.tensor.matmul(
                    TNHp[:, 480 * q : 480 * (q + 1)], uvnhB[:],
                    tb[:, 960 * h + 480 * q : 960 * h + 480 * (q + 1)],
                    start=True, stop=True)
            nc.scalar.activation(TNHs[:, 960 * h : 960 * (h + 1)], TNHp[:],
                                 AF.Copy)

    # ---- interval chains -> LH [128, 1280]: cols (s, ft, i) ----
    TLOv = TLOs[:].rearrange("p (ft k i) -> p ft k i", k=3, i=IMG)
    TNHv = TNHs[:].rearrange("p (ft k i) -> p ft k i", k=3, i=IMG)
    LH = gpool.tile([128, 2 * 640], BF16)
    lo1 = gpool.tile([128, 640], BF16)
    nc.vector.tensor_tensor(lo1[:], TLOv[:, :, 0, :], TLOv[:, :, 1, :], OP.max)
    nc.vector.tensor_tensor(
        LH[:, 0:640].rearrange("p (ft i) -> p ft i", i=IMG),
        lo1[:].rearrange("p (ft i) -> p ft i", i=IMG), TLOv[:, :, 2, :],
        OP.max)
    nh1 = gpool.tile([128, 640], BF16)
    nc.vector.tensor_tensor(nh1[:], TNHv[:, :, 0, :], TNHv[:, :, 1, :], OP.max)
    nh2 = gpool.tile([128, 640], BF16)
    nc.vector.tensor_tensor(
        nh2[:].rearrange("p (ft i) -> p ft i", i=IMG),
        nh1[:].rearrange("p (ft i) -> p ft i", i=IMG), TNHv[:, :, 2, :],
        OP.max)
    # canonicalize empty rows: -hi' = min(-hi, -lo) (point interval)
    nlo = gpool.tile([128, 640], BF16)
    nc.vector.tensor_scalar(nlo[:], LH[:, 0:640], -1.0, None, OP.mult)
    nc.vector.tensor_tensor(LH[:, 640:1280], nh2[:], nlo[:], OP.min)
    LHv = LH[:].rearrange("p (s ft i) -> p s ft i", s=2, ft=NTILE)

    # ---- raster ----
    spool = ctx.enter_context(tc.tile_pool(name="ghp", bufs=3))
    fpool = ctx.enter_context(tc.tile_pool(name="f8p", bufs=1))
    pscnt = ctx.enter_context(tc.tile_pool(name="pcnt", bufs=1, space="PSUM"))
    pdif = ctx.enter_context(tc.tile_pool(name="pdif", bufs=2, space="PSUM"))
    ptd = ctx.enter_context(tc.tile_pool(name="ptd", bufs=2, space="PSUM"))
    pwarm = ctx.enter_context(tc.tile_pool(name="pwarm", bufs=1, space="PSUM"))
    cnt = pscnt.tile([8, 512], F32, tag="cnt")
    wps = pwarm.tile([128, 480], F32, tag="wps")

    def warm(n):
        for _ in range(n):
            nc.tensor.matmul(wps[:], uvloB[:], tb[:, 0:480], start=True,
                             stop=True)

    pair_tiles = {}
    for pi, pr in enumerate(PAIRS):
        t_ = fpool.tile([128, 2 * 2 * NPIX], FP8, tag=f"pair{pi}")
        for hi_, t in enumerate(pr):
            pair_tiles[t] = (t_, hi_, pi)

    NACC = 16 * len(L_TILES) + 16 * len(PAIRS)
    acc_n = [0]

    def accum_flags():
        st = acc_n[0] == 0
        sp = acc_n[0] == NACC - 1
        acc_n[0] += 1
        return st, sp

    def l_accums(mask):
        for c in range(16):
            r = c % 8
            st, sp = accum_flags()
            nc.tensor.matmul(cnt[:], ohb[:, 8 - r : 16 - r],
                             mask[:, 512 * c : 512 * (c + 1)],
                             start=st, stop=sp, skip_group_check=True)

    def dr_accums(pi):
        t_, _, _ = pair_tiles[PAIRS[pi][0]]
        tv = t_[:].rearrange("p (two n) -> p two n", two=2)
        wsel = 32 * pi  # pair 0: weights (1,1); pair 1: (1,2)
        ohv = oh8[:, wsel : wsel + 32].rearrange("p (two w) -> p two w",
                                                 two=2)
        for c in range(16):
            r = c % 8
            st, sp = accum_flags()
            nc.tensor.matmul(cnt[:], ohv[:, :, 8 - r : 16 - r],
                             tv[:, :, 512 * c : 512 * (c + 1)],
                             start=st, stop=sp, skip_group_check=True,
                             perf_mode=mybir.MatmulPerfMode.DoubleRow)

    def compare(t, out):
        lhb = LHv[:, :, t, :].unsqueeze(2).broadcast_to([128, 2, IMG, IMG])
        nc.vector.tensor_tensor(
            out[:].rearrange("p (s j i) -> p s j i", s=2, j=IMG), xxv, lhb,
            OP.is_ge)

    def s_tile(t):
        """PE diff planes + ACT Sign -> fp8 half of a pair tile."""
        t_, hi_, _ = pair_tiles[t]
        base = hi_ * 2 * NPIX
        loP = spool.tile([128, 65], BF16, tag="loP", bufs=2)
        nc.vector.tensor_copy(loP[:, 0:64], LH[:, 64 * t : 64 * (t + 1)])
        nc.vector.memset(loP[:, 64:65], -1.0)
        hiP = spool.tile([128, 65], BF16, tag="hiP", bufs=2)
        nc.vector.tensor_copy(hiP[:, 0:64], LH[:, 640 + 64 * t : 704 + 64 * t])
        nc.vector.memset(hiP[:, 64:65], 1.0)
        lhsT1 = spool.tile([65, 128], BF16, tag="lhsT1", bufs=2)
        lhsT2 = spool.tile([65, 128], BF16, tag="lhsT2", bufs=2)
        loT = ptd.tile([65, 128], BF16, tag="dT")
        nc.tensor.transpose(loT[:], loP[:], idmb[:])
        nc.scalar.activation(lhsT1[:], loT[:], AF.Copy, scale=-1.0)
        hiT = ptd.tile([65, 128], BF16, tag="dT")
        nc.tensor.transpose(hiT[:], hiP[:], idmb[:])
        nc.scalar.activation(lhsT2[:], hiT[:], AF.Copy, scale=-1.0)
        for side, lhsT in ((0, lhsT1), (1, lhsT2)):
            for h in range(4):
                dp = pdif.tile([128, 1024], F32, tag="dp")
                for q in range(2):
                    off = 1024 * h + 512 * q
                    nc.tensor.matmul(dp[:, 512 * q : 512 * (q + 1)], lhsT[:],
                                     xb65[:, off : off + 512], start=True,
                                     stop=True)
                nc.scalar.activation(
                    t_[:, base + 4096 * side + 1024 * h :
                       base + 4096 * side + 1024 * (h + 1)], dp[:], AF.Sign)

    # emission in intended execution order
    for t in range(NTILE):
        if t in CV_TILES:
            mb = spool.tile([128, 2 * NPIX], BF16, tag="ghp")
            compare(t, mb)
            t_, hi_, _ = pair_tiles[t]
            base = hi_ * 2 * NPIX
            nc.gpsimd.tensor_copy(t_[:, base : base + 2 * NPIX], mb[:])
        elif t in S_TILES:
            s_tile(t)
        else:
            mb = spool.tile([128, 2 * NPIX], BF16, tag="ghp")
            compare(t, mb)
            l_accums(mb)
        for pi, pr in enumerate(PAIRS):
            if t == max(pr):
                dr_accums(pi)

    # ---- threshold: covered iff cnt > THR ----
    silb = gpool.tile([8, 512], F32)
    nc.vector.tensor_scalar(silb[:], cnt[:], THR, None, OP.is_gt)
    nc.sync.dma_start(sil_d.ap(), silb[:])


_NC = None


def _get_program():
    global _NC
    if _NC is None:
        nc = bacc.Bacc(
            "TRN2",
            target_bir_lowering=False,
            debug=False,
            enable_asserts=False,
            num_devices=B,
        )
        from contextlib import ExitStack

        with tile.TileContext(nc) as tc:
            with ExitStack() as ctx:
                build_kernel(ctx, tc)
        nc.compile()
        _NC = nc
    return _NC


def _consts():
    """Input-independent constant tables."""
    j = np.arange(IMG, dtype=np.float32)
    xs = (2.0 * j - 63.0) / 64.0                      # exact in bf16
    ys = (63.0 - 2.0 * j) / 64.0
    xg = np.empty((2, IMG, IMG), dtype=np.float32)
    xg[0] = xs[:, None]
    xg[1] = -xs[:, None]
    xgrid = np.broadcast_to(xg.reshape(1, 2 * NPIX), (128, 2 * NPIX))
    xgrid = np.ascontiguousarray(xgrid).astype(ml_dtypes.bfloat16)
    tbv = np.zeros((60, 1920), dtype=np.float32)
    for m in range(30):
        tbv[2 * m, m * 64 : (m + 1) * 64] = ys
        tbv[2 * m + 1, m * 64 : (m + 1) * 64] = 1.0
    tbasis = tbv.astype(ml_dtypes.bfloat16)
    xb = np.zeros((65, NPIX), dtype=np.float32)
    for i in range(IMG):
        xb[i, i::IMG] = 1.0                    # onehot(i) over (j, i) columns
    xb[64] = np.repeat(xs, IMG)                # x_j
    xb65 = xb.astype(ml_dtypes.bfloat16)
    # camera blob: f32 identity + axis-select masks on partitions 0..2
    cblob = np.zeros((128, 132), dtype=np.float32)
    cblob[:, 0:128] = np.eye(128, dtype=np.float32)
    cblob[0, 128] = 1.0
    cblob[1, 129] = 1.0
    cblob[2, 130] = 1.0
    # bf16 sliding onehot (L-path accums, weight 2 at col 8)
    ohb = np.zeros((128, 16), dtype=np.float32)
    ohb[:, 8] = 2.0
    ohb = ohb.astype(ml_dtypes.bfloat16)
    # fp8 DoubleRow onehots: per pair group of 32 cols (two 16-wide halves,
    # weight at col 8 of each half). group 0: (1, 1); group 1: (1, 2).
    oh8 = np.zeros((128, 64), dtype=np.float32)
    oh8[:, 8] = 1.0
    oh8[:, 24] = 1.0
    oh8[:, 40] = 1.0
    oh8[:, 56] = 2.0
    oh8 = oh8.astype(ml_dtypes.float8_e4m3)
    return xgrid, tbasis, xb65, cblob, ohb, oh8


def _host_layout(vertices, faces):
    """Pure indexing: vgt4 [4, 4*NF] where row c, col k*NF + f holds coord c
    (c=3: 1.0) of corner k of face f; corners are (a, b, c, a)."""
    faces4 = np.concatenate([faces, faces[:, :1]], axis=1)  # [NF, 4]
    out = []
    for b in range(B):
        vg = vertices[b][faces4]                      # [NF, 4, 3]
        vg4 = np.concatenate(
            [vg, np.ones((NF, 4, 1), dtype=np.float32)], axis=2)  # [NF,4,4]
        out.append(np.ascontiguousarray(
            vg4.transpose(2, 1, 0).reshape(4, 4 * NF).astype(np.float32)))
    return out


def kernel(vertices, viewpoints, faces, img_size):
    vertices = np.asarray(vertices, dtype=np.float32)
    viewpoints = np.asarray(viewpoints, dtype=np.float32)
    faces = np.asarray(faces, dtype=np.int32)
    assert int(img_size) == IMG and vertices.shape == (B, V, 3)

    nc = _get_program()
    vgts = _host_layout(vertices, faces)
    xgrid, tbasis, xb65, cblob, ohb, oh8 = _consts()
    in_maps = [
        {"vgt4": vgts[b],
         "eye3": np.ascontiguousarray(
             np.broadcast_to(viewpoints[b], (3, 3))).astype(np.float32),
         "cblob": cblob, "ohb": ohb, "oh8": oh8,
         "xgrid": xgrid, "tbasis": tbasis, "xb65": xb65}
        for b in range(B)
    ]
    res = run_bass_kernel_spmd(nc, in_maps, core_ids=list(range(B)))
    # device pixel order is (j, i): transpose back to raster (i, j)
    sil = np.stack([
        res.results[b]["sil"].reshape(IMG, IMG).T for b in range(B)
    ])
    return sil.reshape(B, 1, IMG, IMG).astype(np.float32)


if __name__ == "__main__":
    rng = np.random.default_rng(0)
    verts = rng.standard_normal((B, V, 3), dtype=np.float32) * 0.5
    vps = rng.standard_normal((B, 3), dtype=np.float32)
    fcs = rng.integers(0, V, (NF, 3), dtype=np.int32)
    out = kernel(verts, vps, fcs, IMG)
    print(out.shape, out.sum())


# revision 16
# speedup vs baseline: 1.2907x; 1.2907x over previous
"""Trainium2 Bass kernel for nn_Mesh_Renderer: silhouette via scanline intervals.

Data-parallel over batch (core b renders view b). Host work is layout only
(gather vertices[faces], constant tables, transpose the returned image). All
input-dependent math on device.

Device algorithm (per core):
  1. Camera basis computed with one axis per partition ([3, *] tiles) so the
     projection matrix rt4 = [R^T; -R@eye] [4, 3] is produced by a single PE
     transpose (no SBUF reshape DMA). Projection: 40 K=4 f32 matmuls
     vca[f, (ft, k, d)], perspective divide -> per-corner (xn, yn).
  2. Edge coefficients -> per-edge t-planes t = u*y + v: (u, v) for BOTH the
     lower and negated-upper side packed in one [128, 120] tile, one PE
     transpose -> bf16 lhsT, evaluated against the block-diagonal tbasis by
     PE; drains via ACT (lo) and Pool (nh). Row interval [lo(i), hi(i)] per
     face via DVE max chains; invisible/degenerate faces forced to far-away
     point intervals with +-BIG offsets (sign-sum contributes 0).
  3. Raster, count formulation: cnt(px) = sum_f w_f([x >= lo] + [-x >= -hi]).
     Face tiles split across three pipelines to balance engines:
       L: one DVE is_ge [128, 8192] bf16 -> 16 PE accum matmuls (weight 2).
       S: PE diff-plane matmuls (x - lo, hi - x vs constant xb65 basis) ->
          ACT Sign -> fp8 masks (sign-sum in {0,1,2}, weight 1).
       CV: DVE is_ge -> Pool converts the bf16 masks to fp8 (weight 2).
     fp8 tiles are paired and accumulated with fp8 DoubleRow matmuls
     (2 face tiles per pass, 0.5 cyc/row) into the same PSUM cnt [8, 512].
  4. silhouette = cnt > 2*128*n_step_tiles + 0.5; two-half DMA out; host
     transposes (j, i) -> (i, j).
"""

import sys

if "/opt/trn_rl_repo" not in sys.path:
    sys.path.insert(0, "/opt/trn_rl_repo")

import ml_dtypes
import numpy as np

import concourse.bacc as bacc
import concourse.tile as tile
from concourse import mybir
from concourse.bass_utils import run_bass_kernel_spmd

F32 = mybir.dt.float32
BF16 = mybir.dt.bfloat16
FP8 = mybir.dt.float8e4
I32 = mybir.dt.int32
OP = mybir.AluOpType
AF = mybir.ActivationFunctionType

B, V, NF, IMG = 8, 642, 1280, 64
NPIX = IMG * IMG          # 4096
NTILE = NF // 128         # 10 face tiles
EPS = 1e-8
BIG = 1.0e30
TAN_T = float(np.tan(np.deg2rad(np.float32(15.0)).astype(np.float32)))

# raster pipeline assignment per face tile
S_TILES = (0, 1, 2)          # PE diff + ACT sign -> fp8
CV_TILES = (3,)              # DVE compare + Pool bf16->fp8 convert
PAIRS = ((0, 1), (2, 3))     # DoubleRow pairs (halves in emission order)
L_TILES = tuple(t for t in range(NTILE)
                if t not in S_TILES and t not in CV_TILES)
N_STEP_FACES = 128 * (len(L_TILES) + len(CV_TILES))
THR = 2.0 * N_STEP_FACES + 0.5


def build_kernel(ctx, tc):
    nc = tc.nc
    eye_d = nc.dram_tensor("eye3", [3, 3], F32, kind="ExternalInput")
    vgt_d = nc.dram_tensor("vgt4", [4, 4 * NF], F32, kind="ExternalInput")
    cb_d = nc.dram_tensor("cblob", [128, 132], F32, kind="ExternalInput")
    ohb_d = nc.dram_tensor("ohb", [128, 16], BF16, kind="ExternalInput")
    oh8_d = nc.dram_tensor("oh8", [128, 64], FP8, kind="ExternalInput")
    xb_d = nc.dram_tensor("xb65", [65, NPIX], BF16, kind="ExternalInput")
    tb_d = nc.dram_tensor("tbasis", [60, 1920], BF16, kind="ExternalInput")
    xg_d = nc.dram_tensor("xgrid", [128, 2 * NPIX], BF16, kind="ExternalInput")
    sil_d = nc.dram_tensor("sil", [NPIX], F32, kind="ExternalOutput")

    cpool = ctx.enter_context(tc.tile_pool(name="cam", bufs=1))
    ppool = ctx.enter_context(tc.tile_pool(name="proj", bufs=1))
    gpool = ctx.enter_context(tc.tile_pool(name="grid", bufs=1))

    # ---- input DMAs (emission order = DMA_ENGINES order; big xgrid early
    # but behind everything the compute spine needs) ----
    eye3 = cpool.tile([3, 3], F32)
    nc.sync.dma_start(eye3[:], eye_d.ap())
    vgt = gpool.tile([4, 4 * NF], F32)
    nc.sync.dma_start(vgt[:], vgt_d.ap())
    cb = gpool.tile([128, 132], F32)
    nc.sync.dma_start(cb[:], cb_d.ap())
    ohb = gpool.tile([128, 16], BF16)
    nc.sync.dma_start(ohb[:], ohb_d.ap())
    oh8 = gpool.tile([128, 64], FP8)
    nc.sync.dma_start(oh8[:], oh8_d.ap())
    xb65 = gpool.tile([65, NPIX], BF16)
    nc.sync.dma_start(xb65[:], xb_d.ap())
    tb = gpool.tile([60, 1920], BF16)
    nc.sync.dma_start(tb[:], tb_d.ap())
    xx = gpool.tile([128, 2 * NPIX], BF16)
    nc.sync.dma_start(xx[:], xg_d.ap())
    xxv = xx[:].rearrange("p (s j i) -> p s j i", s=2, j=IMG)

    idm = cb[:, 0:128]                    # f32 identity
    idmb = gpool.tile([128, 128], BF16)
    nc.vector.tensor_copy(idmb[:], idm)

    # ---- camera basis, one axis per partition: p0=x, p1=y, p2=z ----
    nege = cpool.tile([3, 3], F32)
    nc.vector.tensor_scalar(nege[:], eye3[:], -1.0, None, OP.mult)
    # xr = cross(up, -eye) = (nege_z, 0, -nege_x)
    xr = cpool.tile([3, 3], F32)
    nc.vector.memset(xr[:], 0.0)
    nc.vector.tensor_copy(xr[:, 0:1], nege[:, 2:3])
    nc.vector.tensor_scalar(xr[:, 2:3], nege[:, 0:1], -1.0, None, OP.mult)
    # normalize x (DVE) and z (Pool/ACT) in parallel; duplicated [3, 6]
    sqx = cpool.tile([3, 3], F32)
    nc.vector.tensor_tensor(sqx[:], xr[:], xr[:], OP.mult)
    ssx = cpool.tile([3, 1], F32)
    nc.vector.tensor_reduce(ssx[:], sqx[:], mybir.AxisListType.X, OP.add)
    nx_ = cpool.tile([3, 1], F32)
    nc.scalar.activation(nx_[:], ssx[:], AF.Sqrt)
    rx_ = cpool.tile([3, 1], F32)
    nc.vector.reciprocal(rx_[:], nx_[:])
    xn = cpool.tile([3, 6], F32)
    nc.vector.tensor_scalar(
        xn[:].rearrange("p (two d) -> p two d", d=3),
        xr[:].unsqueeze(1).broadcast_to([3, 2, 3]), rx_[:], None, OP.mult)
    sqz = cpool.tile([3, 3], F32)
    nc.gpsimd.tensor_tensor(sqz[:], nege[:], nege[:], OP.mult)
    ssz1 = cpool.tile([3, 1], F32)
    nc.gpsimd.tensor_tensor(ssz1[:], sqz[:, 0:1], sqz[:, 1:2], OP.add)
    ssz = cpool.tile([3, 1], F32)
    nc.gpsimd.tensor_tensor(ssz[:], ssz1[:], sqz[:, 2:3], OP.add)
    nz_ = cpool.tile([3, 1], F32)
    nc.scalar.activation(nz_[:], ssz[:], AF.Sqrt)
    rz_ = cpool.tile([3, 1], F32)
    nc.vector.reciprocal(rz_[:], nz_[:])
    zn = cpool.tile([3, 6], F32)
    nc.vector.tensor_scalar(
        zn[:].rearrange("p (two d) -> p two d", d=3),
        nege[:].unsqueeze(1).broadcast_to([3, 2, 3]), rz_[:], None, OP.mult)
    # y = cross(z, x)
    m1 = cpool.tile([3, 3], F32)
    nc.vector.tensor_tensor(m1[:], zn[:, 1:4], xn[:, 2:5], OP.mult)
    m2 = cpool.tile([3, 3], F32)
    nc.vector.tensor_tensor(m2[:], zn[:, 2:5], xn[:, 1:4], OP.mult)
    y0 = cpool.tile([3, 3], F32)
    nc.vector.tensor_tensor(y0[:], m1[:], m2[:], OP.subtract)
    # per-partition axis select: axes[p] = x*[p==0] + y*[p==1] + z*[p==2]
    t1 = cpool.tile([3, 3], F32)
    nc.vector.tensor_scalar(t1[:], xn[:, 0:3], cb[0:3, 128:129], None, OP.mult)
    t2 = cpool.tile([3, 3], F32)
    nc.vector.tensor_scalar(t2[:], y0[:], cb[0:3, 129:130], None, OP.mult)
    t3 = cpool.tile([3, 3], F32)
    nc.vector.tensor_scalar(t3[:], zn[:, 0:3], cb[0:3, 130:131], None, OP.mult)
    ax12 = cpool.tile([3, 3], F32)
    nc.vector.tensor_tensor(ax12[:], t1[:], t2[:], OP.add)
    axes = cpool.tile([3, 3], F32)
    nc.vector.tensor_tensor(axes[:], ax12[:], t3[:], OP.add)
    # nreye[p] = -dot(eye, axis_p)
    el = cpool.tile([3, 3], F32)
    nc.vector.tensor_tensor(el[:], eye3[:], axes[:], OP.mult)
    dt_ = cpool.tile([3, 1], F32)
    nc.vector.tensor_reduce(dt_[:], el[:], mybir.AxisListType.X, OP.add)
    # rt4T [3, 4]: rows are [axis | -dot]
    rt4T = cpool.tile([3, 4], F32)
    nc.vector.tensor_copy(rt4T[:, 0:3], axes[:])
    nc.vector.tensor_scalar(rt4T[:, 3:4], dt_[:], -1.0, None, OP.mult)
    rt4 = cpool.tile([4, 3], F32)
    with tc.tile_pool(name="ptc", bufs=1, space="PSUM") as ptc:
        rt4p = ptc.tile([4, 3], F32, tag="rt4p")
        nc.tensor.transpose(rt4p[:], rt4T[:], idm[0:3, 0:3])
        nc.vector.tensor_copy(rt4[:], rt4p[:])

    # ---- projection: vca[p, (ft, k, d)] = [w;1]^T @ rt4 per corner ----
    vca = ppool.tile([128, 120], F32)
    with tc.tile_pool(name="pvc", bufs=1, space="PSUM") as psvc:
        vcp = psvc.tile([128, 120], F32)
        for ft in range(NTILE):
            for k in range(4):
                nc.tensor.matmul(
                    vcp[:, 12 * ft + 3 * k : 12 * ft + 3 * (k + 1)],
                    vgt[:, NF * k + 128 * ft : NF * k + 128 * (ft + 1)],
                    rt4[:],
                    start=True,
                    stop=True,
                )
        nc.vector.tensor_copy(vca[:], vcp[:])

    # junk matmuls keep the PE p-state ramp alive between the projection and
    # the T-plane matmuls (idle gaps reset the ramp -> 2-4x matmul cost)
    pwarm = ctx.enter_context(tc.tile_pool(name="pwarm", bufs=1, space="PSUM"))
    wps = pwarm.tile([128, 128], F32, tag="wps")

    def warm(n):
        for _ in range(n):
            nc.tensor.matmul(wps[:], vgt[:, 0:128], vgt[:, 0:128],
                             start=True, stop=True)

    warm(10)

    vcav = vca[:].rearrange("p (c d) -> p c d", d=3)
    vx, vy, vz = vcav[:, :, 0], vcav[:, :, 1], vcav[:, :, 2]

    # perspective divide (raw reciprocal; interval margins tolerate ~3e-3)
    dn = ppool.tile([128, 40], F32)
    nc.vector.tensor_scalar(dn[:], vz, TAN_T, EPS, OP.mult, OP.add)
    rc = ppool.tile([128, 40], F32)
    nc.vector.reciprocal(rc[:], dn[:])
    xn_ = ppool.tile([128, 40], F32)
    nc.vector.tensor_tensor(xn_[:], vx, rc[:], OP.mult)
    yn_ = ppool.tile([128, 40], F32)
    nc.vector.tensor_tensor(yn_[:], vy, rc[:], OP.mult)

    # visibility: all corner z > 0 (on Pool)
    vz4 = vca[:].rearrange("p (ft k d) -> p ft k d", k=4, d=3)
    mz1 = ppool.tile([128, 10], F32)
    nc.vector.tensor_tensor(mz1[:], vz4[:, :, 0, 2], vz4[:, :, 1, 2], OP.min)
    mz = ppool.tile([128, 10], F32)
    nc.vector.tensor_tensor(mz[:], mz1[:], vz4[:, :, 2, 2], OP.min)
    vg = ppool.tile([128, 10], F32)
    nc.vector.tensor_scalar(vg[:], mz[:], 0.0, None, OP.is_gt)

    # ---- edge coefficients [128, 30] in (ft, k) layout ----
    xn4 = xn_[:].rearrange("p (ft k) -> p ft k", k=4)
    yn4 = yn_[:].rearrange("p (ft k) -> p ft k", k=4)
    xk, xk1 = xn4[:, :, 0:3], xn4[:, :, 1:4]
    yk, yk1 = yn4[:, :, 0:3], yn4[:, :, 1:4]

    def t30(name):
        return ppool.tile([128, 30], F32, name=name, tag=name)

    A = t30("A")
    Av = A[:].rearrange("p (ft k) -> p ft k", k=3)
    nc.vector.tensor_tensor(Av, yk, yk1, OP.subtract)
    Bc = t30("Bc")
    Bv = Bc[:].rearrange("p (ft k) -> p ft k", k=3)
    nc.vector.tensor_tensor(Bv, xk1, xk, OP.subtract)
    p1 = t30("p1")
    nc.gpsimd.tensor_tensor(p1[:].rearrange("p (ft k) -> p ft k", k=3), xk,
                            yk1, OP.mult)
    p2 = t30("p2")
    nc.gpsimd.tensor_tensor(p2[:].rearrange("p (ft k) -> p ft k", k=3), yk,
                            xk1, OP.mult)
    C = t30("C")
    nc.gpsimd.tensor_tensor(C[:], p1[:], p2[:], OP.subtract)

    Cv = C[:].rearrange("p (ft k) -> p ft k", k=3)
    S1 = ppool.tile([128, 10], F32, name="S1")
    nc.gpsimd.tensor_tensor(S1[:], Cv[:, :, 0], Cv[:, :, 1], OP.add)
    S = ppool.tile([128, 10], F32, name="S")
    nc.gpsimd.tensor_tensor(S[:], S1[:], Cv[:, :, 2], OP.add)

    # masks (Pool side-chain)
    w = t30("w")
    nc.gpsimd.tensor_tensor(w[:].rearrange("p (ft k) -> p ft k", k=3), Av,
                            S[:].unsqueeze(2).broadcast_to([128, 10, 3]),
                            OP.mult)
    # reciprocal side (DVE)
    iseq = t30("iseq")
    nc.vector.tensor_scalar(iseq[:], A[:], 0.0, None, OP.is_equal)
    Asafe = t30("Asafe")
    nc.vector.tensor_tensor(Asafe[:], A[:], iseq[:], OP.add)
    r0 = t30("r0")
    nc.vector.reciprocal(r0[:], Asafe[:])
    # u = -B/A, v = -C/A; the minus is folded into the mask factors:
    # lower side uses -mpos, upper side uses +mneg (already negated form)
    Bq = t30("Bq")
    nc.vector.tensor_tensor(Bq[:], Bc[:], r0[:], OP.mult)
    Cq = t30("Cq")
    nc.vector.tensor_tensor(Cq[:], C[:], r0[:], OP.mult)
    mpos = t30("mpos")
    nc.vector.tensor_scalar(mpos[:], w[:], 0.0, None, OP.is_gt)
    mposn = t30("mposn")
    nc.vector.tensor_scalar(mposn[:], mpos[:], -1.0, None, OP.mult)
    mneg = t30("mneg")
    nc.vector.tensor_scalar(mneg[:], w[:], 0.0, None, OP.is_lt)
    offlo = t30("offlo")
    nc.vector.tensor_scalar(offlo[:], mpos[:], BIG, -BIG, OP.mult, OP.add)
    offnh = t30("offnh")
    nc.vector.tensor_scalar(offnh[:], mneg[:], BIG, -BIG, OP.mult, OP.add)

    sne = ppool.tile([128, 10], F32, name="sne")
    nc.vector.tensor_scalar(sne[:], S[:], 0.0, None, OP.not_equal)
    visq = ppool.tile([128, 10], F32, name="visq")
    nc.gpsimd.tensor_tensor(visq[:], vg[:], sne[:], OP.mult)
    ivq = ppool.tile([128, 10], F32, name="ivq")
    nc.vector.tensor_scalar(ivq[:], visq[:], -2.0 * BIG, 2.0 * BIG, OP.mult,
                            OP.add)
    ivqN = ppool.tile([128, 10], F32, name="ivqN")
    nc.vector.tensor_scalar(ivqN[:], visq[:], 2.0 * BIG, -2.0 * BIG, OP.mult,
                            OP.add)

    # (u, v) staging for BOTH sides in one [128, 120] tile: cols 0..60 the
    # lower side, 60..120 the negated-upper side, each (m, 2) = (u, v)
    uv12 = ppool.tile([128, 120], F32, name="uv12")
    uvlov = uv12[:, 0:60].rearrange("p (m two) -> p m two", two=2)
    uvnhv = uv12[:, 60:120].rearrange("p (m two) -> p m two", two=2)

    # lower side: ulo = Bq*(-mpos) ; vlo = Cq*(-mpos) + offlo + ivq
    nc.vector.tensor_tensor(uvlov[:, :, 0], Bq[:], mposn[:], OP.mult)
    vlo1 = t30("vlo1")
    nc.vector.tensor_tensor(vlo1[:], Cq[:], mposn[:], OP.mult)
    vlo2 = t30("vlo2")
    nc.vector.tensor_tensor(vlo2[:], vlo1[:], offlo[:], OP.add)
    nc.vector.tensor_tensor(
        uvlov[:, :, 1].rearrange("p (ft k) -> p ft k", k=3),
        vlo2[:].rearrange("p (ft k) -> p ft k", k=3),
        ivq[:].unsqueeze(2).broadcast_to([128, 10, 3]), OP.add)

    # negated upper side: unh = Bq*mneg ; vnh = Cq*mneg + offnh - ivq
    nc.vector.tensor_tensor(uvnhv[:, :, 0], Bq[:], mneg[:], OP.mult)
    vnh1 = t30("vnh1")
    nc.vector.tensor_tensor(vnh1[:], Cq[:], mneg[:], OP.mult)
    vnh2 = t30("vnh2")
    nc.vector.tensor_tensor(vnh2[:], vnh1[:], offnh[:], OP.add)
    nc.vector.tensor_tensor(
        uvnhv[:, :, 1].rearrange("p (ft k) -> p ft k", k=3),
        vnh2[:].rearrange("p (ft k) -> p ft k", k=3),
        ivqN[:].unsqueeze(2).broadcast_to([128, 10, 3]), OP.add)

    # ---- T planes via PE for both sides; drains: lo on ACT, nh on Pool ----
    uvloB = gpool.tile([60, 128], BF16)
    uvnhB = gpool.tile([60, 128], BF16)
    with tc.tile_pool(name="ptr", bufs=2, space="PSUM") as ptr:
        uvloT = ptr.tile([60, 128], F32, tag="uvT")
        nc.tensor.transpose(uvloT[:], uv12[:, 0:60], idm)
        nc.scalar.activation(uvloB[:], uvloT[:], AF.Copy)
        uvnhT = ptr.tile([60, 128], F32, tag="uvT")
        nc.tensor.transpose(uvnhT[:], uv12[:, 60:120], idm)
        nc.scalar.activation(uvnhB[:], uvnhT[:], AF.Copy)
    TLOs = gpool.tile([128, 1920], BF16)
    TNHs = gpool.tile([128, 1920], BF16)
    with tc.tile_pool(name="ptp", bufs=3, space="PSUM") as ptp:
        for h in range(2):
            TLOp = ptp.tile([128, 960], F32, tag="tp")
            for q in range(2):
                nc.tensor.matmul(
                    TLOp[:, 480 * q : 480 * (q + 1)], uvloB[:],
                    tb[:, 960 * h + 480 * q : 960 * h + 480 * (q + 1)],
                    start=True, stop=True)
            nc.scalar.activation(TLOs[:, 960 * h : 960 * (h + 1)], TLOp[:],
                                 AF.Copy)
        for h in range(2):
            TNHp = ptp.tile([128, 960], F32, tag="tp")
            for q in range(2):
                nc.tensor.matmul(
                    TNHp[:, 480 * q : 480 * (q + 1)], uvnhB[:],
                    tb[:, 960 * h + 480 * q : 960 * h + 480 * (q + 1)],
                    start=True, stop=True)
            nc.scalar.activation(TNHs[:, 960 * h : 960 * (h + 1)], TNHp[:],
                                 AF.Copy)

    # ---- interval chains -> LH [128, 1280]: cols (s, ft, i) ----
    TLOv = TLOs[:].rearrange("p (ft k i) -> p ft k i", k=3, i=IMG)
    TNHv = TNHs[:].rearrange("p (ft k i) -> p ft k i", k=3, i=IMG)
    LH = gpool.tile([128, 2 * 640], BF16)
    lo1 = gpool.tile([128, 640], BF16)
    nc.vector.tensor_tensor(lo1[:], TLOv[:, :, 0, :], TLOv[:, :, 1, :], OP.max)
    nc.vector.tensor_tensor(
        LH[:, 0:640].rearrange("p (ft i) -> p ft i", i=IMG),
        lo1[:].rearrange("p (ft i) -> p ft i", i=IMG), TLOv[:, :, 2, :],
        OP.max)
    nh1 = gpool.tile([128, 640], BF16)
    nc.vector.tensor_tensor(nh1[:], TNHv[:, :, 0, :], TNHv[:, :, 1, :], OP.max)
    nh2 = gpool.tile([128, 640], BF16)
    nc.vector.tensor_tensor(
        nh2[:].rearrange("p (ft i) -> p ft i", i=IMG),
        nh1[:].rearrange("p (ft i) -> p ft i", i=IMG), TNHv[:, :, 2, :],
        OP.max)
    # canonicalize empty rows: -hi' = min(-hi, -lo) (point interval)
    nlo = gpool.tile([128, 640], BF16)
    nc.vector.tensor_scalar(nlo[:], LH[:, 0:640], -1.0, None, OP.mult)
    nc.vector.tensor_tensor(LH[:, 640:1280], nh2[:], nlo[:], OP.min)
    LHv = LH[:].rearrange("p (s ft i) -> p s ft i", s=2, ft=NTILE)

    # ---- raster ----
    spool = ctx.enter_context(tc.tile_pool(name="ghp", bufs=3))
    fpool = ctx.enter_context(tc.tile_pool(name="f8p", bufs=1))
    pscnt = ctx.enter_context(tc.tile_pool(name="pcnt", bufs=1, space="PSUM"))
    pdif = ctx.enter_context(tc.tile_pool(name="pdif", bufs=2, space="PSUM"))
    ptd = ctx.enter_context(tc.tile_pool(name="ptd", bufs=2, space="PSUM"))
    cnt = pscnt.tile([8, 512], F32, tag="cnt")

    pair_tiles = {}
    for pi, pr in enumerate(PAIRS):
        t_ = fpool.tile([128, 2 * 2 * NPIX], FP8, tag=f"pair{pi}")
        for hi_, t in enumerate(pr):
            pair_tiles[t] = (t_, hi_, pi)

    NACC = 16 * len(L_TILES) + 16 * len(PAIRS)
    acc_n = [0]

    def accum_flags():
        st = acc_n[0] == 0
        sp = acc_n[0] == NACC - 1
        acc_n[0] += 1
        return st, sp

    def compare(t, out):
        lhb = LHv[:, :, t, :].unsqueeze(2).broadcast_to([128, 2, IMG, IMG])
        nc.vector.tensor_tensor(
            out[:].rearrange("p (s j i) -> p s j i", s=2, j=IMG), xxv, lhb,
            OP.is_ge)

    def l_half(t, mb, h):
        """One side's compare followed by its 8 accum matmuls."""
        lhb = LHv[:, :, t, :].unsqueeze(2).broadcast_to([128, 2, IMG, IMG])
        nc.vector.tensor_tensor(
            mb[:, NPIX * h : NPIX * (h + 1)].rearrange(
                "p (j i) -> p j i", j=IMG),
            xxv[:, h], lhb[:, h], OP.is_ge)
        for c in range(8 * h, 8 * h + 8):
            r = c % 8
            st, sp = accum_flags()
            nc.tensor.matmul(cnt[:], ohb[:, 8 - r : 16 - r],
                             mb[:, 512 * c : 512 * (c + 1)],
                             start=st, stop=sp, skip_group_check=True)

    def dr_accums(pi):
        t_, _, _ = pair_tiles[PAIRS[pi][0]]
        tv = t_[:].rearrange("p (two n) -> p two n", two=2)
        wsel = 32 * pi  # pair 0: weights (1,1); pair 1: (1,2)
        ohv = oh8[:, wsel : wsel + 32].rearrange("p (two w) -> p two w",
                                                 two=2)
        for c in range(16):
            r = c % 8
            st, sp = accum_flags()
            nc.tensor.matmul(cnt[:], ohv[:, :, 8 - r : 16 - r],
                             tv[:, :, 512 * c : 512 * (c + 1)],
                             start=st, stop=sp, skip_group_check=True,
                             perf_mode=mybir.MatmulPerfMode.DoubleRow)

    def s_setup(t):
        """lhsT construction for the PE diff planes of tile t."""
        t_, hi_, _ = pair_tiles[t]
        base = hi_ * 2 * NPIX
        loP = spool.tile([128, 65], BF16, tag="loP", bufs=3)
        nc.vector.tensor_copy(loP[:, 0:64], LH[:, 64 * t : 64 * (t + 1)])
        nc.vector.memset(loP[:, 64:65], -1.0)
        hiP = spool.tile([128, 65], BF16, tag="hiP", bufs=3)
        nc.vector.tensor_copy(hiP[:, 0:64], LH[:, 640 + 64 * t : 704 + 64 * t])
        nc.vector.memset(hiP[:, 64:65], 1.0)
        lhsT1 = spool.tile([65, 128], BF16, tag="lhsT1", bufs=3)
        lhsT2 = spool.tile([65, 128], BF16, tag="lhsT2", bufs=3)
        loT = ptd.tile([65, 128], BF16, tag="dT")
        nc.tensor.transpose(loT[:], loP[:], idmb[:])
        nc.scalar.activation(lhsT1[:], loT[:], AF.Copy, scale=-1.0)
        hiT = ptd.tile([65, 128], BF16, tag="dT")
        nc.tensor.transpose(hiT[:], hiP[:], idmb[:])
        nc.scalar.activation(lhsT2[:], hiT[:], AF.Copy, scale=-1.0)
        return (t_, base, lhsT1, lhsT2)

    def s_group(st_, g):
        """One diff+sign group (1024 px) of an S tile; g in 0..7."""
        t_, base, lhsT1, lhsT2 = st_
        side, h = g // 4, g % 4
        lhsT = lhsT1 if side == 0 else lhsT2
        dp = pdif.tile([128, 1024], F32, tag="dp")
        for q in range(2):
            off = 1024 * h + 512 * q
            nc.tensor.matmul(dp[:, 512 * q : 512 * (q + 1)], lhsT[:],
                             xb65[:, off : off + 512], start=True, stop=True)
        nc.scalar.activation(
            t_[:, base + NPIX * side + 1024 * h :
               base + NPIX * side + 1024 * (h + 1)], dp[:], AF.Sign)

    # --- emission: CV compare+convert first (long Pool pole), S tiles
    # staggered round-robin, L halves interleaved to keep DVE/PE fed ---
    cvt = CV_TILES[0]
    mcv = spool.tile([128, 2 * NPIX], BF16, tag="cvmask", bufs=1)
    compare(cvt, mcv)
    cvd, cvh, _ = pair_tiles[cvt]
    nc.gpsimd.tensor_copy(cvd[:, cvh * 2 * NPIX : (cvh + 1) * 2 * NPIX],
                          mcv[:])
    s_states = {t: s_setup(t) for t in S_TILES}

    l_list = list(L_TILES)
    l_masks = {}
    l_sched = []          # (tile, half) queue
    for t in l_list:
        l_sched.append((t, 0))
        l_sched.append((t, 1))
    li = [0]

    def emit_l_halves(n):
        for _ in range(n):
            if li[0] >= len(l_sched):
                return
            t, h = l_sched[li[0]]
            li[0] += 1
            if h == 0:
                l_masks[t] = spool.tile([128, 2 * NPIX], BF16, tag="ghp",
                                        name=f"lmask{t}")
            l_half(t, l_masks[t], h)

    for g in range(8):
        for t in S_TILES:
            s_group(s_states[t], g)
        emit_l_halves(1 if g < 4 else 2)
    dr_accums(0)
    emit_l_halves(len(l_sched))
    dr_accums(1)

    # ---- threshold: covered iff cnt > THR ----
    silb = gpool.tile([8, 512], F32)
    nc.vector.tensor_scalar(silb[:], cnt[:], THR, None, OP.is_gt)
    nc.sync.dma_start(sil_d.ap(), silb[:])


_NC = None


def _get_program():
    global _NC
    if _NC is None:
        nc = bacc.Bacc(
            "TRN2",
            target_bir_lowering=False,
            debug=False,
            enable_asserts=False,
            num_devices=B,
        )
        from contextlib import ExitStack

        with tile.TileContext(nc) as tc:
            with ExitStack() as ctx:
                build_kernel(ctx, tc)
        nc.compile()
        _NC = nc
    return _NC


def _consts():
    """Input-independent constant tables."""
    j = np.arange(IMG, dtype=np.float32)
    xs = (2.0 * j - 63.0) / 64.0                      # exact in bf16
    ys = (63.0 - 2.0 * j) / 64.0
    xg = np.empty((2, IMG, IMG), dtype=np.float32)
    xg[0] = xs[:, None]
    xg[1] = -xs[:, None]
    xgrid = np.broadcast_to(xg.reshape(1, 2 * NPIX), (128, 2 * NPIX))
    xgrid = np.ascontiguousarray(xgrid).astype(ml_dtypes.bfloat16)
    tbv = np.zeros((60, 1920), dtype=np.float32)
    for m in range(30):
        tbv[2 * m, m * 64 : (m + 1) * 64] = ys
        tbv[2 * m + 1, m * 64 : (m + 1) * 64] = 1.0
    tbasis = tbv.astype(ml_dtypes.bfloat16)
    xb = np.zeros((65, NPIX), dtype=np.float32)
    for i in range(IMG):
        xb[i, i::IMG] = 1.0                    # onehot(i) over (j, i) columns
    xb[64] = np.repeat(xs, IMG)                # x_j
    xb65 = xb.astype(ml_dtypes.bfloat16)
    # camera blob: f32 identity + axis-select masks on partitions 0..2
    cblob = np.zeros((128, 132), dtype=np.float32)
    cblob[:, 0:128] = np.eye(128, dtype=np.float32)
    cblob[0, 128] = 1.0
    cblob[1, 129] = 1.0
    cblob[2, 130] = 1.0
    # bf16 sliding onehot (L-path accums, weight 2 at col 8)
    ohb = np.zeros((128, 16), dtype=np.float32)
    ohb[:, 8] = 2.0
    ohb = ohb.astype(ml_dtypes.bfloat16)
    # fp8 DoubleRow onehots: per pair group of 32 cols (two 16-wide halves,
    # weight at col 8 of each half). group 0: (1, 1); group 1: (1, 2).
    oh8 = np.zeros((128, 64), dtype=np.float32)
    oh8[:, 8] = 1.0
    oh8[:, 24] = 1.0
    oh8[:, 40] = 1.0
    oh8[:, 56] = 2.0
    oh8 = oh8.astype(ml_dtypes.float8_e4m3)
    return xgrid, tbasis, xb65, cblob, ohb, oh8


def _host_layout(vertices, faces):
    """Pure indexing: vgt4 [4, 4*NF] where row c, col k*NF + f holds coord c
    (c=3: 1.0) of corner k of face f; corners are (a, b, c, a)."""
    faces4 = np.concatenate([faces, faces[:, :1]], axis=1)  # [NF, 4]
    out = []
    for b in range(B):
        vg = vertices[b][faces4]                      # [NF, 4, 3]
        vg4 = np.concatenate(
            [vg, np.ones((NF, 4, 1), dtype=np.float32)], axis=2)  # [NF,4,4]
        out.append(np.ascontiguousarray(
            vg4.transpose(2, 1, 0).reshape(4, 4 * NF).astype(np.float32)))
    return out


def kernel(vertices, viewpoints, faces, img_size):
    vertices = np.asarray(vertices, dtype=np.float32)
    viewpoints = np.asarray(viewpoints, dtype=np.float32)
    faces = np.asarray(faces, dtype=np.int32)
    assert int(img_size) == IMG and vertices.shape == (B, V, 3)

    nc = _get_program()
    vgts = _host_layout(vertices, faces)
    xgrid, tbasis, xb65, cblob, ohb, oh8 = _consts()
    in_maps = [
        {"vgt4": vgts[b],
         "eye3": np.ascontiguousarray(
             np.broadcast_to(viewpoints[b], (3, 3))).astype(np.float32),
         "cblob": cblob, "ohb": ohb, "oh8": oh8,
         "xgrid": xgrid, "tbasis": tbasis, "xb65": xb65}
        for b in range(B)
    ]
    res = run_bass_kernel_spmd(nc, in_maps, core_ids=list(range(B)))
    # device pixel order is (j, i): transpose back to raster (i, j)
    sil = np.stack([
        res.results[b]["sil"].reshape(IMG, IMG).T for b in range(B)
    ])
    return sil.reshape(B, 1, IMG, IMG).astype(np.float32)


if __name__ == "__main__":
    rng = np.random.default_rng(0)
    verts = rng.standard_normal((B, V, 3), dtype=np.float32) * 0.5
    vps = rng.standard_normal((B, 3), dtype=np.float32)
    fcs = rng.integers(0, V, (NF, 3), dtype=np.int32)
    out = kernel(verts, vps, fcs, IMG)
    print(out.shape, out.sum())


# revision 22
# speedup vs baseline: 1.3584x; 1.0525x over previous
"""Trainium2 Bass kernel for nn_Mesh_Renderer: silhouette via scanline intervals.

Data-parallel over batch (core b renders view b). Host work is layout only
(gather vertices[faces], constant tables, transpose the returned image). All
input-dependent math on device.

Device algorithm (per core):
  1. Camera basis computed with one axis per partition ([3, *] tiles) so the
     projection matrix rt4 = [R^T; -R@eye] [4, 3] is produced by a single PE
     transpose (no SBUF reshape DMA). Projection: 40 K=4 f32 matmuls
     vca[f, (ft, k, d)], perspective divide -> per-corner (xn, yn).
  2. Edge coefficients -> per-edge t-planes t = u*y + v: (u, v) for BOTH the
     lower and negated-upper side packed in one [128, 120] tile, one PE
     transpose -> bf16 lhsT, evaluated against the block-diagonal tbasis by
     PE; drains via ACT (lo) and Pool (nh). Row interval [lo(i), hi(i)] per
     face via DVE max chains; invisible/degenerate faces forced to far-away
     point intervals with +-BIG offsets (sign-sum contributes 0).
  3. Raster, count formulation: cnt(px) = sum_f w_f([x >= lo] + [-x >= -hi]).
     Face tiles split across three pipelines to balance engines:
       L: one DVE is_ge [128, 8192] bf16 -> 16 PE accum matmuls (weight 2).
       S: PE diff-plane matmuls (x - lo, hi - x vs constant xb65 basis) ->
          ACT Sign -> fp8 masks (sign-sum in {0,1,2}, weight 1).
       CV: DVE is_ge -> Pool converts the bf16 masks to fp8 (weight 2).
     fp8 tiles are paired and accumulated with fp8 DoubleRow matmuls
     (2 face tiles per pass, 0.5 cyc/row) into the same PSUM cnt [8, 512].
  4. silhouette = cnt > 2*128*n_step_tiles + 0.5; two-half DMA out; host
     transposes (j, i) -> (i, j).
"""

import sys

if "/opt/trn_rl_repo" not in sys.path:
    sys.path.insert(0, "/opt/trn_rl_repo")

import ml_dtypes
import numpy as np

import concourse.bacc as bacc
import concourse.tile as tile
from concourse import mybir
from concourse.bass_utils import run_bass_kernel_spmd

F32 = mybir.dt.float32
BF16 = mybir.dt.bfloat16
FP8 = mybir.dt.float8e4
I32 = mybir.dt.int32
OP = mybir.AluOpType
AF = mybir.ActivationFunctionType

B, V, NF, IMG = 8, 642, 1280, 64
NPIX = IMG * IMG          # 4096
NTILE = NF // 128         # 10 face tiles
EPS = 1e-8
BIG = 1.0e30
TAN_T = float(np.tan(np.deg2rad(np.float32(15.0)).astype(np.float32)))

# raster pipeline assignment per face tile
S_TILES = (0, 1, 2)          # PE diff + ACT sign -> fp8
CV_TILES = (3,)              # DVE compare + Pool bf16->fp8 convert
PAIRS = ((0, 1), (2, 3))     # DoubleRow pairs (halves in emission order)
L_TILES = tuple(t for t in range(NTILE)
                if t not in S_TILES and t not in CV_TILES)
N_STEP_FACES = 128 * (len(L_TILES) + len(CV_TILES))
THR = 2.0 * N_STEP_FACES + 0.5


def build_kernel(ctx, tc):
    nc = tc.nc
    eye_d = nc.dram_tensor("eye3", [3, 3], F32, kind="ExternalInput")
    vgt_d = nc.dram_tensor("vgt4", [4, 4 * NF], F32, kind="ExternalInput")
    cb_d = nc.dram_tensor("cblob", [128, 132], F32, kind="ExternalInput")
    ohb_d = nc.dram_tensor("ohb", [128, 16], BF16, kind="ExternalInput")
    oh8_d = nc.dram_tensor("oh8", [128, 64], FP8, kind="ExternalInput")
    xb_d = nc.dram_tensor("xb65", [65, NPIX], BF16, kind="ExternalInput")
    tb_d = nc.dram_tensor("tbasis", [60, 1920], BF16, kind="ExternalInput")
    xg_d = nc.dram_tensor("xgrid", [128, 2 * NPIX], BF16, kind="ExternalInput")
    sil_d = nc.dram_tensor("sil", [NPIX], F32, kind="ExternalOutput")

    cpool = ctx.enter_context(tc.tile_pool(name="cam", bufs=1))
    ppool = ctx.enter_context(tc.tile_pool(name="proj", bufs=1))
    gpool = ctx.enter_context(tc.tile_pool(name="grid", bufs=1))

    # ---- input DMAs (emission order = DMA_ENGINES order; big xgrid early
    # but behind everything the compute spine needs) ----
    eye3 = cpool.tile([3, 3], F32)
    nc.sync.dma_start(eye3[:], eye_d.ap())
    vgt = gpool.tile([4, 4 * NF], F32)
    nc.sync.dma_start(vgt[:], vgt_d.ap())
    cb = gpool.tile([128, 132], F32)
    nc.sync.dma_start(cb[:], cb_d.ap())
    ohb = gpool.tile([128, 16], BF16)
    nc.sync.dma_start(ohb[:], ohb_d.ap())
    oh8 = gpool.tile([128, 64], FP8)
    nc.sync.dma_start(oh8[:], oh8_d.ap())
    xb65 = gpool.tile([65, NPIX], BF16)
    nc.sync.dma_start(xb65[:], xb_d.ap())
    tb = gpool.tile([60, 1920], BF16)
    nc.sync.dma_start(tb[:], tb_d.ap())
    xx = gpool.tile([128, 2 * NPIX], BF16)
    nc.sync.dma_start(xx[:], xg_d.ap())
    xxv = xx[:].rearrange("p (s j i) -> p s j i", s=2, j=IMG)

    idm = cb[:, 0:128]                    # f32 identity
    idmb = gpool.tile([128, 128], BF16)
    nc.vector.tensor_copy(idmb[:], idm)

    # ---- camera basis, one axis per partition: p0=x, p1=y, p2=z ----
    nege = cpool.tile([3, 3], F32)
    nc.vector.tensor_scalar(nege[:], eye3[:], -1.0, None, OP.mult)
    # xr = cross(up, -eye) = (nege_z, 0, -nege_x)
    xr = cpool.tile([3, 3], F32)
    nc.vector.memset(xr[:], 0.0)
    nc.vector.tensor_copy(xr[:, 0:1], nege[:, 2:3])
    nc.vector.tensor_scalar(xr[:, 2:3], nege[:, 0:1], -1.0, None, OP.mult)
    # normalize x (DVE) and z (Pool/ACT) in parallel; duplicated [3, 6]
    sqx = cpool.tile([3, 3], F32)
    nc.vector.tensor_tensor(sqx[:], xr[:], xr[:], OP.mult)
    ssx = cpool.tile([3, 1], F32)
    nc.vector.tensor_reduce(ssx[:], sqx[:], mybir.AxisListType.X, OP.add)
    nx_ = cpool.tile([3, 1], F32)
    nc.scalar.activation(nx_[:], ssx[:], AF.Sqrt)
    rx_ = cpool.tile([3, 1], F32)
    nc.vector.reciprocal(rx_[:], nx_[:])
    xn = cpool.tile([3, 6], F32)
    nc.vector.tensor_scalar(
        xn[:].rearrange("p (two d) -> p two d", d=3),
        xr[:].unsqueeze(1).broadcast_to([3, 2, 3]), rx_[:], None, OP.mult)
    sqz = cpool.tile([3, 3], F32)
    nc.gpsimd.tensor_tensor(sqz[:], nege[:], nege[:], OP.mult)
    ssz1 = cpool.tile([3, 1], F32)
    nc.gpsimd.tensor_tensor(ssz1[:], sqz[:, 0:1], sqz[:, 1:2], OP.add)
    ssz = cpool.tile([3, 1], F32)
    nc.gpsimd.tensor_tensor(ssz[:], ssz1[:], sqz[:, 2:3], OP.add)
    nz_ = cpool.tile([3, 1], F32)
    nc.scalar.activation(nz_[:], ssz[:], AF.Sqrt)
    rz_ = cpool.tile([3, 1], F32)
    nc.vector.reciprocal(rz_[:], nz_[:])
    zn = cpool.tile([3, 6], F32)
    nc.vector.tensor_scalar(
        zn[:].rearrange("p (two d) -> p two d", d=3),
        nege[:].unsqueeze(1).broadcast_to([3, 2, 3]), rz_[:], None, OP.mult)
    # y = cross(z, x)
    m1 = cpool.tile([3, 3], F32)
    nc.vector.tensor_tensor(m1[:], zn[:, 1:4], xn[:, 2:5], OP.mult)
    m2 = cpool.tile([3, 3], F32)
    nc.vector.tensor_tensor(m2[:], zn[:, 2:5], xn[:, 1:4], OP.mult)
    y0 = cpool.tile([3, 3], F32)
    nc.vector.tensor_tensor(y0[:], m1[:], m2[:], OP.subtract)
    # per-partition axis select: axes[p] = x*[p==0] + y*[p==1] + z*[p==2]
    t1 = cpool.tile([3, 3], F32)
    nc.vector.tensor_scalar(t1[:], xn[:, 0:3], cb[0:3, 128:129], None, OP.mult)
    t2 = cpool.tile([3, 3], F32)
    nc.vector.tensor_scalar(t2[:], y0[:], cb[0:3, 129:130], None, OP.mult)
    t3 = cpool.tile([3, 3], F32)
    nc.vector.tensor_scalar(t3[:], zn[:, 0:3], cb[0:3, 130:131], None, OP.mult)
    ax12 = cpool.tile([3, 3], F32)
    nc.vector.tensor_tensor(ax12[:], t1[:], t2[:], OP.add)
    axes = cpool.tile([3, 3], F32)
    nc.vector.tensor_tensor(axes[:], ax12[:], t3[:], OP.add)
    # nreye[p] = -dot(eye, axis_p)
    el = cpool.tile([3, 3], F32)
    nc.vector.tensor_tensor(el[:], eye3[:], axes[:], OP.mult)
    dt_ = cpool.tile([3, 1], F32)
    nc.vector.tensor_reduce(dt_[:], el[:], mybir.AxisListType.X, OP.add)
    # rt4T [3, 4]: rows are [axis | -dot]
    rt4T = cpool.tile([3, 4], F32)
    nc.vector.tensor_copy(rt4T[:, 0:3], axes[:])
    nc.vector.tensor_scalar(rt4T[:, 3:4], dt_[:], -1.0, None, OP.mult)
    rt4 = cpool.tile([4, 3], F32)
    with tc.tile_pool(name="ptc", bufs=1, space="PSUM") as ptc:
        rt4p = ptc.tile([4, 3], F32, tag="rt4p")
        nc.tensor.transpose(rt4p[:], rt4T[:], idm[0:3, 0:3])
        nc.vector.tensor_copy(rt4[:], rt4p[:])

    # ---- projection: vca[p, (ft, k, d)] = [w;1]^T @ rt4 per corner ----
    vca = ppool.tile([128, 120], F32)
    with tc.tile_pool(name="pvc", bufs=1, space="PSUM") as psvc:
        vcp = psvc.tile([128, 120], F32)
        for ft in range(NTILE):
            for k in range(4):
                nc.tensor.matmul(
                    vcp[:, 12 * ft + 3 * k : 12 * ft + 3 * (k + 1)],
                    vgt[:, NF * k + 128 * ft : NF * k + 128 * (ft + 1)],
                    rt4[:],
                    start=True,
                    stop=True,
                )
        nc.vector.tensor_copy(vca[:], vcp[:])

    # junk matmuls keep the PE p-state ramp alive between the projection and
    # the T-plane matmuls (idle gaps reset the ramp -> 2-4x matmul cost)
    pwarm = ctx.enter_context(tc.tile_pool(name="pwarm", bufs=1, space="PSUM"))
    wps = pwarm.tile([128, 128], F32, tag="wps")

    def warm(n):
        for _ in range(n):
            nc.tensor.matmul(wps[:], vgt[:, 0:128], vgt[:, 0:128],
                             start=True, stop=True)

    warm(8)

    vcav = vca[:].rearrange("p (c d) -> p c d", d=3)
    vx, vy, vz = vcav[:, :, 0], vcav[:, :, 1], vcav[:, :, 2]

    # perspective divide (raw reciprocal; interval margins tolerate ~3e-3)
    dn = ppool.tile([128, 40], F32)
    nc.vector.tensor_scalar(dn[:], vz, TAN_T, EPS, OP.mult, OP.add)
    rc = ppool.tile([128, 40], F32)
    nc.vector.reciprocal(rc[:], dn[:])
    xn_ = ppool.tile([128, 40], F32)
    nc.vector.tensor_tensor(xn_[:], vx, rc[:], OP.mult)
    yn_ = ppool.tile([128, 40], F32)
    nc.vector.tensor_tensor(yn_[:], vy, rc[:], OP.mult)

    # visibility: all corner z > 0 (on Pool)
    vz4 = vca[:].rearrange("p (ft k d) -> p ft k d", k=4, d=3)
    mz1 = ppool.tile([128, 10], F32)
    nc.vector.tensor_tensor(mz1[:], vz4[:, :, 0, 2], vz4[:, :, 1, 2], OP.min)
    mz = ppool.tile([128, 10], F32)
    nc.vector.tensor_tensor(mz[:], mz1[:], vz4[:, :, 2, 2], OP.min)
    vg = ppool.tile([128, 10], F32)
    nc.vector.tensor_scalar(vg[:], mz[:], 0.0, None, OP.is_gt)

    # ---- edge coefficients [128, 30] in (ft, k) layout ----
    xn4 = xn_[:].rearrange("p (ft k) -> p ft k", k=4)
    yn4 = yn_[:].rearrange("p (ft k) -> p ft k", k=4)
    xk, xk1 = xn4[:, :, 0:3], xn4[:, :, 1:4]
    yk, yk1 = yn4[:, :, 0:3], yn4[:, :, 1:4]

    def t30(name):
        return ppool.tile([128, 30], F32, name=name, tag=name)

    A = t30("A")
    Av = A[:].rearrange("p (ft k) -> p ft k", k=3)
    nc.vector.tensor_tensor(Av, yk, yk1, OP.subtract)
    Bc = t30("Bc")
    Bv = Bc[:].rearrange("p (ft k) -> p ft k", k=3)
    nc.vector.tensor_tensor(Bv, xk1, xk, OP.subtract)
    p1 = t30("p1")
    nc.gpsimd.tensor_tensor(p1[:].rearrange("p (ft k) -> p ft k", k=3), xk,
                            yk1, OP.mult)
    p2 = t30("p2")
    nc.gpsimd.tensor_tensor(p2[:].rearrange("p (ft k) -> p ft k", k=3), yk,
                            xk1, OP.mult)
    C = t30("C")
    nc.gpsimd.tensor_tensor(C[:], p1[:], p2[:], OP.subtract)

    Cv = C[:].rearrange("p (ft k) -> p ft k", k=3)
    S1 = ppool.tile([128, 10], F32, name="S1")
    nc.gpsimd.tensor_tensor(S1[:], Cv[:, :, 0], Cv[:, :, 1], OP.add)
    S = ppool.tile([128, 10], F32, name="S")
    nc.gpsimd.tensor_tensor(S[:], S1[:], Cv[:, :, 2], OP.add)

    # masks (Pool side-chain)
    w = t30("w")
    nc.gpsimd.tensor_tensor(w[:].rearrange("p (ft k) -> p ft k", k=3), Av,
                            S[:].unsqueeze(2).broadcast_to([128, 10, 3]),
                            OP.mult)
    # reciprocal side (DVE)
    iseq = t30("iseq")
    nc.vector.tensor_scalar(iseq[:], A[:], 0.0, None, OP.is_equal)
    Asafe = t30("Asafe")
    nc.vector.tensor_tensor(Asafe[:], A[:], iseq[:], OP.add)
    r0 = t30("r0")
    nc.vector.reciprocal(r0[:], Asafe[:])
    # u = -B/A, v = -C/A; the minus is folded into the mask factors:
    # lower side uses -mpos, upper side uses +mneg (already negated form)
    Bq = t30("Bq")
    nc.vector.tensor_tensor(Bq[:], Bc[:], r0[:], OP.mult)
    Cq = t30("Cq")
    nc.vector.tensor_tensor(Cq[:], C[:], r0[:], OP.mult)
    mpos = t30("mpos")
    nc.vector.tensor_scalar(mpos[:], w[:], 0.0, None, OP.is_gt)
    mposn = t30("mposn")
    nc.vector.tensor_scalar(mposn[:], mpos[:], -1.0, None, OP.mult)
    mneg = t30("mneg")
    nc.vector.tensor_scalar(mneg[:], w[:], 0.0, None, OP.is_lt)
    offlo = t30("offlo")
    nc.vector.tensor_scalar(offlo[:], mpos[:], BIG, -BIG, OP.mult, OP.add)
    offnh = t30("offnh")
    nc.vector.tensor_scalar(offnh[:], mneg[:], BIG, -BIG, OP.mult, OP.add)

    sne = ppool.tile([128, 10], F32, name="sne")
    nc.vector.tensor_scalar(sne[:], S[:], 0.0, None, OP.not_equal)
    visq = ppool.tile([128, 10], F32, name="visq")
    nc.gpsimd.tensor_tensor(visq[:], vg[:], sne[:], OP.mult)
    ivq = ppool.tile([128, 10], F32, name="ivq")
    nc.vector.tensor_scalar(ivq[:], visq[:], -2.0 * BIG, 2.0 * BIG, OP.mult,
                            OP.add)
    ivqN = ppool.tile([128, 10], F32, name="ivqN")
    nc.vector.tensor_scalar(ivqN[:], visq[:], 2.0 * BIG, -2.0 * BIG, OP.mult,
                            OP.add)

    # (u, v) staging for BOTH sides in one [128, 120] tile: cols 0..60 the
    # lower side, 60..120 the negated-upper side, each (m, 2) = (u, v)
    uv12 = ppool.tile([128, 120], F32, name="uv12")
    uvlov = uv12[:, 0:60].rearrange("p (m two) -> p m two", two=2)
    uvnhv = uv12[:, 60:120].rearrange("p (m two) -> p m two", two=2)

    # lower side: ulo = Bq*(-mpos) ; vlo = Cq*(-mpos) + offlo + ivq
    nc.vector.tensor_tensor(uvlov[:, :, 0], Bq[:], mposn[:], OP.mult)
    vlo1 = t30("vlo1")
    nc.vector.tensor_tensor(vlo1[:], Cq[:], mposn[:], OP.mult)
    vlo2 = t30("vlo2")
    nc.vector.tensor_tensor(vlo2[:], vlo1[:], offlo[:], OP.add)
    nc.vector.tensor_tensor(
        uvlov[:, :, 1].rearrange("p (ft k) -> p ft k", k=3),
        vlo2[:].rearrange("p (ft k) -> p ft k", k=3),
        ivq[:].unsqueeze(2).broadcast_to([128, 10, 3]), OP.add)

    # negated upper side: unh = Bq*mneg ; vnh = Cq*mneg + offnh - ivq
    nc.vector.tensor_tensor(uvnhv[:, :, 0], Bq[:], mneg[:], OP.mult)
    vnh1 = t30("vnh1")
    nc.vector.tensor_tensor(vnh1[:], Cq[:], mneg[:], OP.mult)
    vnh2 = t30("vnh2")
    nc.vector.tensor_tensor(vnh2[:], vnh1[:], offnh[:], OP.add)
    nc.vector.tensor_tensor(
        uvnhv[:, :, 1].rearrange("p (ft k) -> p ft k", k=3),
        vnh2[:].rearrange("p (ft k) -> p ft k", k=3),
        ivqN[:].unsqueeze(2).broadcast_to([128, 10, 3]), OP.add)

    # ---- T planes via PE for both sides; drains: lo on ACT, nh on Pool ----
    uvloB = gpool.tile([60, 128], BF16)
    uvnhB = gpool.tile([60, 128], BF16)
    with tc.tile_pool(name="ptr", bufs=2, space="PSUM") as ptr:
        uvloT = ptr.tile([60, 128], F32, tag="uvT")
        nc.tensor.transpose(uvloT[:], uv12[:, 0:60], idm)
        nc.scalar.activation(uvloB[:], uvloT[:], AF.Copy)
        uvnhT = ptr.tile([60, 128], F32, tag="uvT")
        nc.tensor.transpose(uvnhT[:], uv12[:, 60:120], idm)
        nc.scalar.activation(uvnhB[:], uvnhT[:], AF.Copy)
    # T matmuls + drains + interval max chains, processed per half (h0 =
    # face tiles 0..4, h1 = 5..9) so the raster can start on h0's tiles
    # while h1 is still draining.
    TLOs = gpool.tile([128, 1920], BF16)
    TNHs = gpool.tile([128, 1920], BF16)
    LH = gpool.tile([128, 2 * 640], BF16)
    lo1 = gpool.tile([128, 640], BF16)
    nh1 = gpool.tile([128, 640], BF16)
    nh2 = gpool.tile([128, 640], BF16)
    nlo = gpool.tile([128, 640], BF16)
    TLOv = TLOs[:].rearrange("p (ft k i) -> p ft k i", k=3, i=IMG)
    TNHv = TNHs[:].rearrange("p (ft k i) -> p ft k i", k=3, i=IMG)
    lo1v = lo1[:].rearrange("p (ft i) -> p ft i", i=IMG)
    LHlov = LH[:, 0:640].rearrange("p (ft i) -> p ft i", i=IMG)
    nh1v = nh1[:].rearrange("p (ft i) -> p ft i", i=IMG)
    nh2v = nh2[:].rearrange("p (ft i) -> p ft i", i=IMG)
    with tc.tile_pool(name="ptp", bufs=3, space="PSUM") as ptp:
        for h in range(2):
            f0, f1 = 5 * h, 5 * (h + 1)
            c0, c1 = 960 * h, 960 * (h + 1)
            TLOp = ptp.tile([128, 960], F32, tag="tp", name=f"TLOp{h}")
            for q in range(2):
                nc.tensor.matmul(
                    TLOp[:, 480 * q : 480 * (q + 1)], uvloB[:],
                    tb[:, c0 + 480 * q : c0 + 480 * (q + 1)],
                    start=True, stop=True)
            nc.scalar.activation(TLOs[:, c0:c1], TLOp[:], AF.Copy)
            TNHp = ptp.tile([128, 960], F32, tag="tp", name=f"TNHp{h}")
            for q in range(2):
                nc.tensor.matmul(
                    TNHp[:, 480 * q : 480 * (q + 1)], uvnhB[:],
                    tb[:, c0 + 480 * q : c0 + 480 * (q + 1)],
                    start=True, stop=True)
            nc.scalar.activation(TNHs[:, c0:c1], TNHp[:], AF.Copy)
            # interval max chains for this half's 5 face tiles
            nc.vector.tensor_tensor(lo1[:, 320 * h : 320 * (h + 1)],
                                    TLOv[:, f0:f1, 0, :],
                                    TLOv[:, f0:f1, 1, :], OP.max)
            nc.vector.tensor_tensor(LHlov[:, f0:f1], lo1v[:, f0:f1],
                                    TLOv[:, f0:f1, 2, :], OP.max)
            nc.vector.tensor_scalar(nlo[:, 320 * h : 320 * (h + 1)],
                                    LH[:, 320 * h : 320 * (h + 1)], -1.0,
                                    None, OP.mult)
            nc.vector.tensor_tensor(nh1[:, 320 * h : 320 * (h + 1)],
                                    TNHv[:, f0:f1, 0, :],
                                    TNHv[:, f0:f1, 1, :], OP.max)
            nc.vector.tensor_tensor(nh2v[:, f0:f1], nh1v[:, f0:f1],
                                    TNHv[:, f0:f1, 2, :], OP.max)
            # canonicalize empty rows: -hi' = min(-hi, -lo) (point interval)
            nc.vector.tensor_tensor(
                LH[:, 640 + 320 * h : 640 + 320 * (h + 1)],
                nh2[:, 320 * h : 320 * (h + 1)],
                nlo[:, 320 * h : 320 * (h + 1)], OP.min)
    LHv = LH[:].rearrange("p (s ft i) -> p s ft i", s=2, ft=NTILE)

    # ---- raster ----
    spool = ctx.enter_context(tc.tile_pool(name="ghp", bufs=3))
    fpool = ctx.enter_context(tc.tile_pool(name="f8p", bufs=1))
    pscnt = ctx.enter_context(tc.tile_pool(name="pcnt", bufs=1, space="PSUM"))
    pdif = ctx.enter_context(tc.tile_pool(name="pdif", bufs=2, space="PSUM"))
    ptd = ctx.enter_context(tc.tile_pool(name="ptd", bufs=2, space="PSUM"))
    cnt = pscnt.tile([8, 512], F32, tag="cnt")

    pair_tiles = {}
    for pi, pr in enumerate(PAIRS):
        t_ = fpool.tile([128, 2 * 2 * NPIX], FP8, tag=f"pair{pi}")
        for hi_, t in enumerate(pr):
            pair_tiles[t] = (t_, hi_, pi)

    NACC = 16 * len(L_TILES) + 16 * len(PAIRS)
    acc_n = [0]

    def accum_flags():
        st = acc_n[0] == 0
        sp = acc_n[0] == NACC - 1
        acc_n[0] += 1
        return st, sp

    def compare(t, out):
        lhb = LHv[:, :, t, :].unsqueeze(2).broadcast_to([128, 2, IMG, IMG])
        nc.vector.tensor_tensor(
            out[:].rearrange("p (s j i) -> p s j i", s=2, j=IMG), xxv, lhb,
            OP.is_ge)

    def l_half(t, mb, h):
        """One side's compare followed by its 8 accum matmuls."""
        lhb = LHv[:, :, t, :].unsqueeze(2).broadcast_to([128, 2, IMG, IMG])
        nc.vector.tensor_tensor(
            mb[:, NPIX * h : NPIX * (h + 1)].rearrange(
                "p (j i) -> p j i", j=IMG),
            xxv[:, h], lhb[:, h], OP.is_ge)
        for c in range(8 * h, 8 * h + 8):
            r = c % 8
            st, sp = accum_flags()
            nc.tensor.matmul(cnt[:], ohb[:, 8 - r : 16 - r],
                             mb[:, 512 * c : 512 * (c + 1)],
                             start=st, stop=sp, skip_group_check=True)

    def dr_accums(pi):
        t_, _, _ = pair_tiles[PAIRS[pi][0]]
        tv = t_[:].rearrange("p (two n) -> p two n", two=2)
        wsel = 32 * pi  # pair 0: weights (1,1); pair 1: (1,2)
        ohv = oh8[:, wsel : wsel + 32].rearrange("p (two w) -> p two w",
                                                 two=2)
        for c in range(16):
            r = c % 8
            st, sp = accum_flags()
            nc.tensor.matmul(cnt[:], ohv[:, :, 8 - r : 16 - r],
                             tv[:, :, 512 * c : 512 * (c + 1)],
                             start=st, stop=sp, skip_group_check=True,
                             perf_mode=mybir.MatmulPerfMode.DoubleRow)

    def s_setup(t):
        """lhsT construction for the PE diff planes of tile t."""
        t_, hi_, _ = pair_tiles[t]
        base = hi_ * 2 * NPIX
        loP = spool.tile([128, 65], BF16, tag="loP", bufs=3)
        nc.vector.tensor_copy(loP[:, 0:64], LH[:, 64 * t : 64 * (t + 1)])
        nc.vector.memset(loP[:, 64:65], -1.0)
        hiP = spool.tile([128, 65], BF16, tag="hiP", bufs=3)
        nc.vector.tensor_copy(hiP[:, 0:64], LH[:, 640 + 64 * t : 704 + 64 * t])
        nc.vector.memset(hiP[:, 64:65], 1.0)
        lhsT1 = spool.tile([65, 128], BF16, tag="lhsT1", bufs=3)
        lhsT2 = spool.tile([65, 128], BF16, tag="lhsT2", bufs=3)
        loT = ptd.tile([65, 128], BF16, tag="dT")
        nc.tensor.transpose(loT[:], loP[:], idmb[:])
        nc.scalar.activation(lhsT1[:], loT[:], AF.Copy, scale=-1.0)
        hiT = ptd.tile([65, 128], BF16, tag="dT")
        nc.tensor.transpose(hiT[:], hiP[:], idmb[:])
        nc.scalar.activation(lhsT2[:], hiT[:], AF.Copy, scale=-1.0)
        return (t_, base, lhsT1, lhsT2)

    def s_group(st_, g):
        """One diff+sign group (1024 px) of an S tile; g in 0..7."""
        t_, base, lhsT1, lhsT2 = st_
        side, h = g // 4, g % 4
        lhsT = lhsT1 if side == 0 else lhsT2
        dp = pdif.tile([128, 1024], F32, tag="dp")
        for q in range(2):
            off = 1024 * h + 512 * q
            nc.tensor.matmul(dp[:, 512 * q : 512 * (q + 1)], lhsT[:],
                             xb65[:, off : off + 512], start=True, stop=True)
        nc.scalar.activation(
            t_[:, base + NPIX * side + 1024 * h :
               base + NPIX * side + 1024 * (h + 1)], dp[:], AF.Sign)

    # --- emission: CV compare+convert first (long Pool pole), S tiles
    # staggered round-robin, L halves interleaved to keep DVE/PE fed ---
    cvt = CV_TILES[0]
    mcv = spool.tile([128, 2 * NPIX], BF16, tag="cvmask", bufs=1)
    compare(cvt, mcv)
    cvd, cvh, _ = pair_tiles[cvt]
    nc.gpsimd.tensor_copy(cvd[:, cvh * 2 * NPIX : (cvh + 1) * 2 * NPIX],
                          mcv[:])
    s_states = {t: s_setup(t) for t in S_TILES}

    l_list = list(L_TILES)
    l_masks = {}
    l_sched = []          # (tile, half) queue
    for t in l_list:
        l_sched.append((t, 0))
        l_sched.append((t, 1))
    li = [0]

    def emit_l_halves(n):
        for _ in range(n):
            if li[0] >= len(l_sched):
                return
            t, h = l_sched[li[0]]
            li[0] += 1
            if h == 0:
                l_masks[t] = spool.tile([128, 2 * NPIX], BF16, tag="ghp",
                                        name=f"lmask{t}")
            l_half(t, l_masks[t], h)

    # t-major S groups: pair halves complete early so DR accums overlap;
    # one L half interleaved per 4 S groups keeps DVE/PE fed. The final
    # L halves run last so the stop-flagged accum is cheap and early.
    for si, t in enumerate(S_TILES):
        for g in range(4):
            s_group(s_states[t], g)
        emit_l_halves(1)
        for g in range(4, 8):
            s_group(s_states[t], g)
        emit_l_halves(1)
        if t == PAIRS[0][1]:
            dr_accums(0)
    dr_accums(1)
    emit_l_halves(len(l_sched))

    # ---- threshold: covered iff cnt > THR ----
    silb = gpool.tile([8, 512], F32)
    nc.vector.tensor_scalar(silb[:], cnt[:], THR, None, OP.is_gt)
    nc.sync.dma_start(sil_d.ap(), silb[:])


_NC = None


def _get_program():
    global _NC
    if _NC is None:
        nc = bacc.Bacc(
            "TRN2",
            target_bir_lowering=False,
            debug=False,
            enable_asserts=False,
            num_devices=B,
        )
        from contextlib import ExitStack

        with tile.TileContext(nc) as tc:
            with ExitStack() as ctx:
                build_kernel(ctx, tc)
        nc.compile()
        _NC = nc
    return _NC


def _consts():
    """Input-independent constant tables."""
    j = np.arange(IMG, dtype=np.float32)
    xs = (2.0 * j - 63.0) / 64.0                      # exact in bf16
    ys = (63.0 - 2.0 * j) / 64.0
    xg = np.empty((2, IMG, IMG), dtype=np.float32)
    xg[0] = xs[:, None]
    xg[1] = -xs[:, None]
    xgrid = np.broadcast_to(xg.reshape(1, 2 * NPIX), (128, 2 * NPIX))
    xgrid = np.ascontiguousarray(xgrid).astype(ml_dtypes.bfloat16)
    tbv = np.zeros((60, 1920), dtype=np.float32)
    for m in range(30):
        tbv[2 * m, m * 64 : (m + 1) * 64] = ys
        tbv[2 * m + 1, m * 64 : (m + 1) * 64] = 1.0
    tbasis = tbv.astype(ml_dtypes.bfloat16)
    xb = np.zeros((65, NPIX), dtype=np.float32)
    for i in range(IMG):
        xb[i, i::IMG] = 1.0                    # onehot(i) over (j, i) columns
    xb[64] = np.repeat(xs, IMG)                # x_j
    xb65 = xb.astype(ml_dtypes.bfloat16)
    # camera blob: f32 identity + axis-select masks on partitions 0..2
    cblob = np.zeros((128, 132), dtype=np.float32)
    cblob[:, 0:128] = np.eye(128, dtype=np.float32)
    cblob[0, 128] = 1.0
    cblob[1, 129] = 1.0
    cblob[2, 130] = 1.0
    # bf16 sliding onehot (L-path accums, weight 2 at col 8)
    ohb = np.zeros((128, 16), dtype=np.float32)
    ohb[:, 8] = 2.0
    ohb = ohb.astype(ml_dtypes.bfloat16)
    # fp8 DoubleRow onehots: per pair group of 32 cols (two 16-wide halves,
    # weight at col 8 of each half). group 0: (1, 1); group 1: (1, 2).
    oh8 = np.zeros((128, 64), dtype=np.float32)
    oh8[:, 8] = 1.0
    oh8[:, 24] = 1.0
    oh8[:, 40] = 1.0
    oh8[:, 56] = 2.0
    oh8 = oh8.astype(ml_dtypes.float8_e4m3)
    return xgrid, tbasis, xb65, cblob, ohb, oh8


def _host_layout(vertices, faces):
    """Pure indexing: vgt4 [4, 4*NF] where row c, col k*NF + f holds coord c
    (c=3: 1.0) of corner k of face f; corners are (a, b, c, a)."""
    faces4 = np.concatenate([faces, faces[:, :1]], axis=1)  # [NF, 4]
    out = []
    for b in range(B):
        vg = vertices[b][faces4]                      # [NF, 4, 3]
        vg4 = np.concatenate(
            [vg, np.ones((NF, 4, 1), dtype=np.float32)], axis=2)  # [NF,4,4]
        out.append(np.ascontiguousarray(
            vg4.transpose(2, 1, 0).reshape(4, 4 * NF).astype(np.float32)))
    return out


def kernel(vertices, viewpoints, faces, img_size):
    vertices = np.asarray(vertices, dtype=np.float32)
    viewpoints = np.asarray(viewpoints, dtype=np.float32)
    faces = np.asarray(faces, dtype=np.int32)
    assert int(img_size) == IMG and vertices.shape == (B, V, 3)

    nc = _get_program()
    vgts = _host_layout(vertices, faces)
    xgrid, tbasis, xb65, cblob, ohb, oh8 = _consts()
    in_maps = [
        {"vgt4": vgts[b],
         "eye3": np.ascontiguousarray(
             np.broadcast_to(viewpoints[b], (3, 3))).astype(np.float32),
         "cblob": cblob, "ohb": ohb, "oh8": oh8,
         "xgrid": xgrid, "tbasis": tbasis, "xb65": xb65}
        for b in range(B)
    ]
    res = run_bass_kernel_spmd(nc, in_maps, core_ids=list(range(B)))
    # device pixel order is (j, i): transpose back to raster (i, j)
    sil = np.stack([
        res.results[b]["sil"].reshape(IMG, IMG).T for b in range(B)
    ])
    return sil.reshape(B, 1, IMG, IMG).astype(np.float32)


if __name__ == "__main__":
    rng = np.random.default_rng(0)
    verts = rng.standard_normal((B, V, 3), dtype=np.float32) * 0.5
    vps = rng.standard_normal((B, 3), dtype=np.float32)
    fcs = rng.integers(0, V, (NF, 3), dtype=np.int32)
    out = kernel(verts, vps, fcs, IMG)
    print(out.shape, out.sum())
